# revision 22
# baseline (speedup 1.0000x reference)
"""Conv2d 3x3 (B=32, 256->256 ch, 64x64, pad 1) on 8 trn2 NeuronCores.

Data-parallel over batch: 4 images per core, weight/bias replicated.

Two algorithms:
- 'direct': implicit GEMM, 9 taps x 2 ci-blocks of shifted matmuls
  accumulating in PSUM (bf16 inputs, fp32 accumulation).
- 'wino': 1-D Winograd F(2,3) along W. PE work drops 1.5x (6 effective
  taps instead of 9). Host deinterleaves the padded image into 4
  aligned column planes (xe, xo, xe1, xo1) so the DVE input transform
  runs in 2x packed-bf16 mode; the 4 Winograd m-slots accumulate in 4
  PSUM banks per group; the inverse transform is 2 ACT ops (m0+bias /
  -m3+bias, reading PSUM) + 4 in-place DVE adds.
"""

import numpy as np

B, CIN, COUT, H, W, KS = 32, 256, 256, 64, 64, 3
NCORES = 8
BPC = B // NCORES            # images per core
CB = CIN // 128              # input-channel blocks
QB = COUT // 128             # output-channel blocks

ALGO = "wino"

_prog = None


def _make_nc():
    from concourse import bacc

    return bacc.Bacc("TRN2", target_bir_lowering=False, debug=False)


def build_direct():
    import concourse.mybir as mybir
    from concourse.tile import TileContext

    bf16 = mybir.dt.bfloat16
    f32 = mybir.dt.float32
    NROW = 8                     # output rows per matmul group (N = 512)
    NRB = H // NROW

    nc = _make_nc()
    x_d = nc.dram_tensor("x", [BPC, 128, CB, H + 2, W + 2], bf16,
                         kind="ExternalInput").ap()
    w_d = nc.dram_tensor("w", [128, CB, KS, KS, QB, 128], bf16,
                         kind="ExternalInput").ap()
    b_d = nc.dram_tensor("bias", [128, QB], f32, kind="ExternalInput").ap()
    o_d = nc.dram_tensor("out", [BPC, QB, 128, H, W], f32, kind="ExternalOutput").ap()

    with TileContext(nc) as tc:
        with tc.tile_pool(name="const", bufs=1) as cpool, \
             tc.tile_pool(name="xpad", bufs=3) as xpool, \
             tc.tile_pool(name="ot", bufs=4) as opool, \
             tc.tile_pool(name="ps", bufs=4, space="PSUM") as ppool:
            # image 0 load first (critical path), chunked over rows
            xps = []
            row_chunks = [(0, 18), (18, 34), (34, 50), (50, 66)]
            xp0 = xpool.tile([128, CB, H + 2, W + 2], bf16, tag="xp")
            for (r0, r1) in row_chunks:
                nc.sync.dma_start(out=xp0[:, :, r0:r1, :],
                                  in_=x_d[0, :, :, r0:r1, :])
            xps.append(xp0)

            w_sb = cpool.tile([128, CB, KS, KS, QB, 128], bf16)
            nc.sync.dma_start(out=w_sb[:], in_=w_d[:])
            bias_sb = cpool.tile([128, QB], f32)
            nc.sync.dma_start(out=bias_sb[:], in_=b_d[:])

            for b in range(BPC):
                if b < len(xps):
                    xp = xps[b]
                else:
                    xp = xpool.tile([128, CB, H + 2, W + 2], bf16, tag="xp")
                    nc.sync.dma_start(out=xp[:], in_=x_d[b])

                for cb in range(QB):
                    for rb in range(NRB):
                        y0 = rb * NROW
                        ps = ppool.tile([128, NROW * W], f32)
                        n_mm = CB * KS * KS
                        i = 0
                        for ib in range(CB):
                            for kh in range(KS):
                                for kw in range(KS):
                                    nc.tensor.matmul(
                                        ps[:],
                                        lhsT=w_sb[:, ib, kh, kw, cb, :],
                                        rhs=xp[:, ib, y0 + kh:y0 + kh + NROW,
                                               kw:kw + W],
                                        start=(i == 0),
                                        stop=(i == n_mm - 1),
                                    )
                                    i += 1
                        ot = opool.tile([128, NROW * W], f32)
                        nc.vector.tensor_scalar_add(ot[:], ps[:],
                                                    bias_sb[:, cb:cb + 1])
                        nc.sync.dma_start(out=o_d[b, cb, :, y0:y0 + NROW, :],
                                          in_=ot[:])
    nc.compile()
    return nc


def build_wino():
    import concourse.mybir as mybir
    from concourse.tile import TileContext

    bf16 = mybir.dt.bfloat16
    f32 = mybir.dt.float32
    NROW = 16                    # output rows per group (N = 16*32 = 512)
    NRB = H // NROW              # 4 groups per (img, co-blk)
    NT = W // 2                  # 32 Winograd tiles per row
    HP = H + 2                   # padded rows

    nc = _make_nc()
    # 4 column planes per (img, blk): xe, xo, xe1, xo1 -- each [66 rows, 32]
    x_d = nc.dram_tensor("x", [BPC, 128, CB, 4, HP, NT], bf16,
                         kind="ExternalInput").ap()
    # Winograd-transformed weights U[p], p=0..3
    w_d = nc.dram_tensor("w", [128, QB, CB, KS, 4, 128], bf16,
                         kind="ExternalInput").ap()
    b_d = nc.dram_tensor("bias", [128, QB], f32, kind="ExternalInput").ap()
    o_d = nc.dram_tensor("out", [BPC, QB, 128, H, W], f32, kind="ExternalOutput").ap()

    def transform_ops(nc, v_sb, xeo, ranges):
        # planes: 0=xe, 1=xo, 2=xe1, 3=xo1. One thunk per DVE op so the
        # caller can meter them out between matmul groups.
        ops = []
        for (r0, r1) in ranges:
            rs = slice(r0, r1)
            for ib in range(CB):
                xe = xeo[:, ib, 0, rs]
                xo = xeo[:, ib, 1, rs]
                xe1 = xeo[:, ib, 2, rs]
                xo1 = xeo[:, ib, 3, rs]
                ops += [
                    lambda o=v_sb[:, ib, 0, rs], a=xe, b=xe1:
                        nc.vector.tensor_sub(o, a, b),          # V0=d0-d2
                    lambda o=v_sb[:, ib, 1, rs], a=xo, b=xe1:
                        nc.vector.tensor_add(o, a, b),          # V1=d1+d2
                    lambda o=v_sb[:, ib, 2, rs], a=xe1, b=xo:
                        nc.vector.tensor_sub(o, a, b),          # V2=d2-d1
                    lambda o=v_sb[:, ib, 3, rs], a=xo, b=xo1:
                        nc.vector.tensor_sub(o, a, b),          # V3=d1-d3
                ]
        return ops

    def transform(nc, v_sb, xeo, r0, r1):
        for op in transform_ops(nc, v_sb, xeo, [(r0, r1)]):
            op()

    with TileContext(nc) as tc:
        with tc.tile_pool(name="const", bufs=1) as cpool, \
             tc.tile_pool(name="xeo", bufs=2) as xpool, \
             tc.tile_pool(name="vv", bufs=2) as vpool, \
             tc.tile_pool(name="ot", bufs=3) as opool, \
             tc.tile_pool(name="ps", bufs=2, space="PSUM") as ppool:
            # image 0: DMA + transform in row chunks to shorten the
            # critical path to the first matmul group
            chunks = [(0, 18), (18, 34), (34, 50), (50, 66)]
            # weights first: cb=0 half gates the very first matmul group
            w_sb = cpool.tile([128, QB, CB, KS, 4, 128], bf16)
            nc.sync.dma_start(out=w_sb[:, 0], in_=w_d[:, 0])
            bias_sb = cpool.tile([128, QB], f32)
            nc.sync.dma_start(out=bias_sb[:], in_=b_d[:])

            # image 0: fine-grained chunks up front to minimize the
            # DMA->transform->first-matmul critical path
            chunks0 = [(0, 9), (9, 18), (18, 34), (34, 50), (50, 66)]
            xeo0 = xpool.tile([128, CB, 4, HP, NT], bf16, tag="xeo")
            v0 = vpool.tile([128, CB, 4, HP, NT], bf16, tag="vv")
            r0, r1 = chunks0[0]
            nc.sync.dma_start(out=xeo0[:, :, :, r0:r1, :],
                              in_=x_d[0, :, :, :, r0:r1, :])
            nc.sync.dma_start(out=w_sb[:, 1], in_=w_d[:, 1])
            for (r0, r1) in chunks0[1:]:
                nc.sync.dma_start(out=xeo0[:, :, :, r0:r1, :],
                                  in_=x_d[0, :, :, :, r0:r1, :])
            for (r0, r1) in chunks0:
                transform(nc, v0, xeo0, r0, r1)

            vs = [v0]
            for b in range(BPC):
                v_sb = vs[b]
                # prefetch next image's planes; its transform is emitted in
                # chunks BETWEEN this image's groups (below) so it fills DVE
                # idle time without delaying PSUM-draining inverse ops
                if b + 1 < BPC:
                    xeo_n = xpool.tile([128, CB, 4, HP, NT], bf16, tag="xeo")
                    nc.sync.dma_start(out=xeo_n[:], in_=x_d[b + 1])
                    v_n = vpool.tile([128, CB, 4, HP, NT], bf16, tag="vv")
                    vs.append(v_n)
                    next_ops = transform_ops(nc, v_n, xeo_n, chunks)
                else:
                    next_ops = []

                def emit_group(cb, y0, nrow):
                    ps = ppool.tile([128, 4, NROW * NT], f32, tag="ps")
                    # pos order 0,3,1,2: both ACT seeds (m0, m3) complete
                    # early so the inverse chains overlap the m1/m2 matmuls
                    for p in (0, 3, 1, 2):
                        i = 0
                        for ib in range(CB):
                            for kh in range(KS):
                                nc.tensor.matmul(
                                    ps[:, p, :nrow * NT],
                                    lhsT=w_sb[:, cb, ib, kh, p, :],
                                    rhs=v_sb[:, ib, p, y0 + kh:y0 + kh + nrow, :],
                                    start=(i == 0),
                                    stop=(i == CB * KS - 1),
                                )
                                i += 1
                    ot = opool.tile([128, NROW, W], f32, tag="ot")
                    ot = ot[:, :nrow]
                    ev = ot[:, :, 0:W:2]
                    od = ot[:, :, 1:W:2]
                    m = [ps[:, p, :nrow * NT]
                         .rearrange("p (r t) -> p r t", t=NT) for p in range(4)]
                    # Y_even = m0+m1+m2+bias ; Y_odd = m1-m2-m3+bias
                    nc.scalar.activation(
                        ev, m[0], mybir.ActivationFunctionType.Identity,
                        bias=bias_sb[:, cb:cb + 1], scale=1.0)
                    nc.scalar.activation(
                        od, m[3], mybir.ActivationFunctionType.Identity,
                        bias=bias_sb[:, cb:cb + 1], scale=-1.0)
                    nc.vector.tensor_add(ev, ev, m[1])
                    nc.vector.tensor_add(ev, ev, m[2])
                    nc.vector.tensor_add(od, od, m[1])
                    nc.vector.tensor_sub(od, od, m[2])
                    nc.sync.dma_start(out=o_d[b, cb, :, y0:y0 + nrow, :],
                                      in_=ot[:])

                first = (b == 0)
                last = (b == BPC - 1)
                for cb in range(QB):
                    for rb in range(NRB):
                        y0 = rb * NROW
                        if first and cb == 0 and rb == 0:
                            # short first groups: compute starts sooner
                            emit_group(cb, 0, NROW // 2)
                            emit_group(cb, NROW // 2, NROW // 2)
                        elif last and cb == QB - 1 and rb == NRB - 1:
                            # short last groups: faster pipeline drain
                            emit_group(cb, y0, NROW // 2)
                            emit_group(cb, y0 + NROW // 2, NROW // 2)
                        else:
                            emit_group(cb, y0, NROW)
                        for _ in range(4):
                            if next_ops:
                                next_ops.pop(0)()
    nc.compile()
    return nc


def prep_direct(x, weight, bias):
    import ml_dtypes
    xs = x.reshape(NCORES, BPC, CB, 128, H, W).transpose(0, 1, 3, 2, 4, 5)
    xs = np.pad(xs, [(0, 0), (0, 0), (0, 0), (0, 0), (1, 1), (1, 1)])
    xs = np.ascontiguousarray(xs).astype(ml_dtypes.bfloat16)
    # w[co, ci, kh, kw] -> [ci, ci_blk, kh, kw, co_blk, co]
    wp = weight.reshape(QB, 128, CB, 128, KS, KS).transpose(3, 2, 4, 5, 0, 1)
    wp = np.ascontiguousarray(wp).astype(ml_dtypes.bfloat16)
    bp = np.ascontiguousarray(bias.reshape(QB, 128).T).astype(np.float32)
    return [{"x": xs[c], "w": wp, "bias": bp} for c in range(NCORES)]


def prep_wino(x, weight, bias):
    import ml_dtypes
    NT = W // 2
    xs = x.reshape(NCORES, BPC, CB, 128, H, W).transpose(0, 1, 3, 2, 4, 5)
    xs = np.pad(xs, [(0, 0)] * 4 + [(1, 1), (1, 1)])  # [NC,BPC,128,CB,66,66]
    xe = xs[..., 0::2]                                 # cols 0,2,..,64  (33)
    xo = xs[..., 1::2]                                 # cols 1,3,..,65  (33)
    planes = np.stack([xe[..., 0:NT], xo[..., 0:NT],
                       xe[..., 1:NT + 1], xo[..., 1:NT + 1]], axis=4)
    # [NC, BPC, 128, CB, 4, 66, NT]
    xp = np.ascontiguousarray(planes).astype(ml_dtypes.bfloat16)
    # U[p] from g=w[..,kh,:]: U0=g0, U1=(g0+g1+g2)/2, U2=(g0-g1+g2)/2, U3=g2
    g = weight.astype(np.float64)
    u = np.stack([g[..., 0],
                  (g[..., 0] + g[..., 1] + g[..., 2]) * 0.5,
                  (g[..., 0] - g[..., 1] + g[..., 2]) * 0.5,
                  g[..., 2]], axis=-1)                 # [co, ci, kh, 4]
    up = u.reshape(QB, 128, CB, 128, KS, 4).transpose(3, 0, 2, 4, 5, 1)
    up = np.ascontiguousarray(up).astype(ml_dtypes.bfloat16)
    bp = np.ascontiguousarray(bias.reshape(QB, 128).T).astype(np.float32)
    return [{"x": xp[c], "w": up, "bias": bp} for c in range(NCORES)]


def build_program():
    return build_wino() if ALGO == "wino" else build_direct()


def prep_inputs(x, weight, bias):
    return (prep_wino if ALGO == "wino" else prep_direct)(x, weight, bias)


def kernel(x, weight, bias):
    global _prog
    from concourse.bass_utils import run_bass_kernel_spmd

    if _prog is None:
        _prog = build_program()
    in_maps = prep_inputs(np.asarray(x, dtype=np.float32),
                          np.asarray(weight, dtype=np.float32),
                          np.asarray(bias, dtype=np.float32))
    res = run_bass_kernel_spmd(_prog, in_maps, list(range(NCORES)))
    outs = [r["out"].reshape(BPC, COUT, H, W) for r in res.results]
    return np.concatenate(outs, axis=0).astype(np.float32)


# revision 23
# speedup vs baseline: 1.0078x; 1.0078x over previous
"""Conv2d 3x3 (B=32, 256->256 ch, 64x64, pad 1) on 8 trn2 NeuronCores.

Data-parallel over batch: 4 images per core, weight/bias replicated.

Two algorithms:
- 'direct': implicit GEMM, 9 taps x 2 ci-blocks of shifted matmuls
  accumulating in PSUM (bf16 inputs, fp32 accumulation).
- 'wino': 1-D Winograd F(2,3) along W. PE work drops 1.5x (6 effective
  taps instead of 9). Host deinterleaves the padded image into 4
  aligned column planes (xe, xo, xe1, xo1) so the DVE input transform
  runs in 2x packed-bf16 mode; the 4 Winograd m-slots accumulate in 4
  PSUM banks per group; the inverse transform is 2 ACT ops (m0+bias /
  -m3+bias, reading PSUM) + 4 in-place DVE adds.
"""

import numpy as np

B, CIN, COUT, H, W, KS = 32, 256, 256, 64, 64, 3
NCORES = 8
BPC = B // NCORES            # images per core
CB = CIN // 128              # input-channel blocks
QB = COUT // 128             # output-channel blocks

ALGO = "wino"

_prog = None


def _make_nc():
    from concourse import bacc

    return bacc.Bacc("TRN2", target_bir_lowering=False, debug=False)


def build_direct():
    import concourse.mybir as mybir
    from concourse.tile import TileContext

    bf16 = mybir.dt.bfloat16
    f32 = mybir.dt.float32
    NROW = 8                     # output rows per matmul group (N = 512)
    NRB = H // NROW

    nc = _make_nc()
    x_d = nc.dram_tensor("x", [BPC, 128, CB, H + 2, W + 2], bf16,
                         kind="ExternalInput").ap()
    w_d = nc.dram_tensor("w", [128, CB, KS, KS, QB, 128], bf16,
                         kind="ExternalInput").ap()
    b_d = nc.dram_tensor("bias", [128, QB], f32, kind="ExternalInput").ap()
    o_d = nc.dram_tensor("out", [BPC, QB, 128, H, W], f32, kind="ExternalOutput").ap()

    with TileContext(nc) as tc:
        with tc.tile_pool(name="const", bufs=1) as cpool, \
             tc.tile_pool(name="xpad", bufs=3) as xpool, \
             tc.tile_pool(name="ot", bufs=4) as opool, \
             tc.tile_pool(name="ps", bufs=4, space="PSUM") as ppool:
            # image 0 load first (critical path), chunked over rows
            xps = []
            row_chunks = [(0, 18), (18, 34), (34, 50), (50, 66)]
            xp0 = xpool.tile([128, CB, H + 2, W + 2], bf16, tag="xp")
            for (r0, r1) in row_chunks:
                nc.sync.dma_start(out=xp0[:, :, r0:r1, :],
                                  in_=x_d[0, :, :, r0:r1, :])
            xps.append(xp0)

            w_sb = cpool.tile([128, CB, KS, KS, QB, 128], bf16)
            nc.sync.dma_start(out=w_sb[:], in_=w_d[:])
            bias_sb = cpool.tile([128, QB], f32)
            nc.sync.dma_start(out=bias_sb[:], in_=b_d[:])

            for b in range(BPC):
                if b < len(xps):
                    xp = xps[b]
                else:
                    xp = xpool.tile([128, CB, H + 2, W + 2], bf16, tag="xp")
                    nc.sync.dma_start(out=xp[:], in_=x_d[b])

                for cb in range(QB):
                    for rb in range(NRB):
                        y0 = rb * NROW
                        ps = ppool.tile([128, NROW * W], f32)
                        n_mm = CB * KS * KS
                        i = 0
                        for ib in range(CB):
                            for kh in range(KS):
                                for kw in range(KS):
                                    nc.tensor.matmul(
                                        ps[:],
                                        lhsT=w_sb[:, ib, kh, kw, cb, :],
                                        rhs=xp[:, ib, y0 + kh:y0 + kh + NROW,
                                               kw:kw + W],
                                        start=(i == 0),
                                        stop=(i == n_mm - 1),
                                    )
                                    i += 1
                        ot = opool.tile([128, NROW * W], f32)
                        nc.vector.tensor_scalar_add(ot[:], ps[:],
                                                    bias_sb[:, cb:cb + 1])
                        nc.sync.dma_start(out=o_d[b, cb, :, y0:y0 + NROW, :],
                                          in_=ot[:])
    nc.compile()
    return nc


def build_wino():
    import concourse.mybir as mybir
    from concourse.tile import TileContext

    bf16 = mybir.dt.bfloat16
    f32 = mybir.dt.float32
    NROW = 16                    # output rows per group (N = 16*32 = 512)
    NRB = H // NROW              # 4 groups per (img, co-blk)
    NT = W // 2                  # 32 Winograd tiles per row
    HP = H + 2                   # padded rows

    nc = _make_nc()
    # 4 column planes per (img, blk): xe, xo, xe1, xo1 -- each [66 rows, 32]
    x_d = nc.dram_tensor("x", [BPC, 128, CB, 4, HP, NT], bf16,
                         kind="ExternalInput").ap()
    # Winograd-transformed weights U[p], p=0..3
    w_d = nc.dram_tensor("w", [128, QB, CB, KS, 4, 128], bf16,
                         kind="ExternalInput").ap()
    b_d = nc.dram_tensor("bias", [128, QB], f32, kind="ExternalInput").ap()
    o_d = nc.dram_tensor("out", [BPC, QB, 128, H, W], f32, kind="ExternalOutput").ap()

    def transform_ops(nc, v_sb, xeo, ranges):
        # planes: 0=xe, 1=xo, 2=xe1, 3=xo1. One thunk per DVE op so the
        # caller can meter them out between matmul groups.
        ops = []
        for (r0, r1) in ranges:
            rs = slice(r0, r1)
            for ib in range(CB):
                xe = xeo[:, ib, 0, rs]
                xo = xeo[:, ib, 1, rs]
                xe1 = xeo[:, ib, 2, rs]
                xo1 = xeo[:, ib, 3, rs]
                ops += [
                    lambda o=v_sb[:, ib, 0, rs], a=xe, b=xe1:
                        nc.vector.tensor_sub(o, a, b),          # V0=d0-d2
                    lambda o=v_sb[:, ib, 1, rs], a=xo, b=xe1:
                        nc.vector.tensor_add(o, a, b),          # V1=d1+d2
                    lambda o=v_sb[:, ib, 2, rs], a=xe1, b=xo:
                        nc.vector.tensor_sub(o, a, b),          # V2=d2-d1
                    lambda o=v_sb[:, ib, 3, rs], a=xo, b=xo1:
                        nc.vector.tensor_sub(o, a, b),          # V3=d1-d3
                ]
        return ops

    def transform(nc, v_sb, xeo, r0, r1):
        for op in transform_ops(nc, v_sb, xeo, [(r0, r1)]):
            op()

    with TileContext(nc) as tc:
        with tc.tile_pool(name="const", bufs=1) as cpool, \
             tc.tile_pool(name="xeo", bufs=2) as xpool, \
             tc.tile_pool(name="vv", bufs=2) as vpool, \
             tc.tile_pool(name="ot", bufs=3) as opool, \
             tc.tile_pool(name="ps", bufs=2, space="PSUM") as ppool:
            # image 0: DMA + transform in row chunks to shorten the
            # critical path to the first matmul group
            chunks = [(0, 18), (18, 34), (34, 50), (50, 66)]
            # weights first: cb=0 half gates the very first matmul group
            w_sb = cpool.tile([128, QB, CB, KS, 4, 128], bf16)
            nc.sync.dma_start(out=w_sb[:, 0], in_=w_d[:, 0])
            bias_sb = cpool.tile([128, QB], f32)
            nc.sync.dma_start(out=bias_sb[:], in_=b_d[:])

            # image 0: fine-grained chunks up front to minimize the
            # DMA->transform->first-matmul critical path
            chunks0 = [(0, 9), (9, 18), (18, 34), (34, 50), (50, 66)]
            xeo0 = xpool.tile([128, CB, 4, HP, NT], bf16, tag="xeo")
            v0 = vpool.tile([128, CB, 4, HP, NT], bf16, tag="vv")
            r0, r1 = chunks0[0]
            nc.sync.dma_start(out=xeo0[:, :, :, r0:r1, :],
                              in_=x_d[0, :, :, :, r0:r1, :])
            nc.sync.dma_start(out=w_sb[:, 1], in_=w_d[:, 1])
            for (r0, r1) in chunks0[1:]:
                nc.sync.dma_start(out=xeo0[:, :, :, r0:r1, :],
                                  in_=x_d[0, :, :, :, r0:r1, :])
            for (r0, r1) in chunks0[:2]:
                transform(nc, v0, xeo0, r0, r1)

            vs = [v0]
            xeo = xeo0
            for b in range(BPC):
                v_sb = vs[b]
                # prefetch next image's planes; transform ops are metered
                # out between groups (~5/group) so DVE stays just under PE
                pend = transform_ops(nc, v_sb, xeo, [(18, 34), (34, 50),
                                                     (50, 66)]) \
                    if b == 0 else pend_next
                if b + 1 < BPC:
                    xeo_n = xpool.tile([128, CB, 4, HP, NT], bf16, tag="xeo")
                    nc.sync.dma_start(out=xeo_n[:], in_=x_d[b + 1])
                    v_n = vpool.tile([128, CB, 4, HP, NT], bf16, tag="vv")
                    vs.append(v_n)
                    pend += transform_ops(nc, v_n, xeo_n, [(0, 9), (9, 18)])
                    pend_next = transform_ops(nc, v_n, xeo_n,
                                              [(18, 34), (34, 50), (50, 66)])
                    xeo = xeo_n

                def emit_group(cb, y0, nrow):
                    ps = ppool.tile([128, 4, NROW * NT], f32, tag="ps")
                    # pos order 0,3,1,2: both ACT seeds (m0, m3) complete
                    # early so the inverse chains overlap the m1/m2 matmuls
                    for p in (0, 3, 1, 2):
                        i = 0
                        for ib in range(CB):
                            for kh in range(KS):
                                nc.tensor.matmul(
                                    ps[:, p, :nrow * NT],
                                    lhsT=w_sb[:, cb, ib, kh, p, :],
                                    rhs=v_sb[:, ib, p, y0 + kh:y0 + kh + nrow, :],
                                    start=(i == 0),
                                    stop=(i == CB * KS - 1),
                                )
                                i += 1
                    ot = opool.tile([128, NROW, W], f32, tag="ot")
                    ot = ot[:, :nrow]
                    ev = ot[:, :, 0:W:2]
                    od = ot[:, :, 1:W:2]
                    m = [ps[:, p, :nrow * NT]
                         .rearrange("p (r t) -> p r t", t=NT) for p in range(4)]
                    # Y_even = m0+m1+m2+bias ; Y_odd = m1-m2-m3+bias
                    nc.scalar.activation(
                        ev, m[0], mybir.ActivationFunctionType.Identity,
                        bias=bias_sb[:, cb:cb + 1], scale=1.0)
                    nc.scalar.activation(
                        od, m[3], mybir.ActivationFunctionType.Identity,
                        bias=bias_sb[:, cb:cb + 1], scale=-1.0)
                    nc.vector.tensor_add(ev, ev, m[1])
                    nc.vector.tensor_add(ev, ev, m[2])
                    nc.vector.tensor_add(od, od, m[1])
                    nc.vector.tensor_sub(od, od, m[2])
                    nc.sync.dma_start(out=o_d[b, cb, :, y0:y0 + nrow, :],
                                      in_=ot[:])

                first = (b == 0)
                last = (b == BPC - 1)
                # (rb, cb) order: V row-chunks are consumed progressively,
                # so metered transforms always land a group ahead of use
                for rb in range(NRB):
                    for cb in range(QB):
                        y0 = rb * NROW
                        if first and rb == 0 and cb == 0:
                            emit_group(cb, 0, NROW // 2)
                            for op in pend[:3]:
                                op()
                            del pend[:3]
                            emit_group(cb, NROW // 2, NROW // 2)
                        elif last and rb == NRB - 1 and cb == QB - 1:
                            emit_group(cb, y0, NROW // 2)
                            emit_group(cb, y0 + NROW // 2, NROW // 2)
                        else:
                            emit_group(cb, y0, NROW)
                        n = min(5, len(pend))
                        for op in pend[:n]:
                            op()
                        del pend[:n]
                for op in pend:
                    op()
    nc.compile()
    return nc


def prep_direct(x, weight, bias):
    import ml_dtypes
    xs = x.reshape(NCORES, BPC, CB, 128, H, W).transpose(0, 1, 3, 2, 4, 5)
    xs = np.pad(xs, [(0, 0), (0, 0), (0, 0), (0, 0), (1, 1), (1, 1)])
    xs = np.ascontiguousarray(xs).astype(ml_dtypes.bfloat16)
    # w[co, ci, kh, kw] -> [ci, ci_blk, kh, kw, co_blk, co]
    wp = weight.reshape(QB, 128, CB, 128, KS, KS).transpose(3, 2, 4, 5, 0, 1)
    wp = np.ascontiguousarray(wp).astype(ml_dtypes.bfloat16)
    bp = np.ascontiguousarray(bias.reshape(QB, 128).T).astype(np.float32)
    return [{"x": xs[c], "w": wp, "bias": bp} for c in range(NCORES)]


def prep_wino(x, weight, bias):
    import ml_dtypes
    NT = W // 2
    xs = x.reshape(NCORES, BPC, CB, 128, H, W).transpose(0, 1, 3, 2, 4, 5)
    xs = np.pad(xs, [(0, 0)] * 4 + [(1, 1), (1, 1)])  # [NC,BPC,128,CB,66,66]
    xe = xs[..., 0::2]                                 # cols 0,2,..,64  (33)
    xo = xs[..., 1::2]                                 # cols 1,3,..,65  (33)
    planes = np.stack([xe[..., 0:NT], xo[..., 0:NT],
                       xe[..., 1:NT + 1], xo[..., 1:NT + 1]], axis=4)
    # [NC, BPC, 128, CB, 4, 66, NT]
    xp = np.ascontiguousarray(planes).astype(ml_dtypes.bfloat16)
    # U[p] from g=w[..,kh,:]: U0=g0, U1=(g0+g1+g2)/2, U2=(g0-g1+g2)/2, U3=g2
    g = weight.astype(np.float64)
    u = np.stack([g[..., 0],
                  (g[..., 0] + g[..., 1] + g[..., 2]) * 0.5,
                  (g[..., 0] - g[..., 1] + g[..., 2]) * 0.5,
                  g[..., 2]], axis=-1)                 # [co, ci, kh, 4]
    up = u.reshape(QB, 128, CB, 128, KS, 4).transpose(3, 0, 2, 4, 5, 1)
    up = np.ascontiguousarray(up).astype(ml_dtypes.bfloat16)
    bp = np.ascontiguousarray(bias.reshape(QB, 128).T).astype(np.float32)
    return [{"x": xp[c], "w": up, "bias": bp} for c in range(NCORES)]


def build_program():
    return build_wino() if ALGO == "wino" else build_direct()


def prep_inputs(x, weight, bias):
    return (prep_wino if ALGO == "wino" else prep_direct)(x, weight, bias)


def kernel(x, weight, bias):
    global _prog
    from concourse.bass_utils import run_bass_kernel_spmd

    if _prog is None:
        _prog = build_program()
    in_maps = prep_inputs(np.asarray(x, dtype=np.float32),
                          np.asarray(weight, dtype=np.float32),
                          np.asarray(bias, dtype=np.float32))
    res = run_bass_kernel_spmd(_prog, in_maps, list(range(NCORES)))
    outs = [r["out"].reshape(BPC, COUT, H, W) for r in res.results]
    return np.concatenate(outs, axis=0).astype(np.float32)


# revision 24
# speedup vs baseline: 1.0573x; 1.0491x over previous
"""Conv2d 3x3 (B=32, 256->256 ch, 64x64, pad 1) on 8 trn2 NeuronCores.

Data-parallel over batch: 4 images per core, weight/bias replicated.

Two algorithms:
- 'direct': implicit GEMM, 9 taps x 2 ci-blocks of shifted matmuls
  accumulating in PSUM (bf16 inputs, fp32 accumulation).
- 'wino': 1-D Winograd F(2,3) along W. PE work drops 1.5x (6 effective
  taps instead of 9). Host deinterleaves the padded image into 4
  aligned column planes (xe, xo, xe1, xo1) so the DVE input transform
  runs in 2x packed-bf16 mode; the 4 Winograd m-slots accumulate in 4
  PSUM banks per group; the inverse transform is 2 ACT ops (m0+bias /
  -m3+bias, reading PSUM) + 4 in-place DVE adds.
"""

import numpy as np

B, CIN, COUT, H, W, KS = 32, 256, 256, 64, 64, 3
NCORES = 8
BPC = B // NCORES            # images per core
CB = CIN // 128              # input-channel blocks
QB = COUT // 128             # output-channel blocks

ALGO = "wino"

_prog = None


def _make_nc():
    from concourse import bacc

    return bacc.Bacc("TRN2", target_bir_lowering=False, debug=False)


def build_direct():
    import concourse.mybir as mybir
    from concourse.tile import TileContext

    bf16 = mybir.dt.bfloat16
    f32 = mybir.dt.float32
    NROW = 8                     # output rows per matmul group (N = 512)
    NRB = H // NROW

    nc = _make_nc()
    x_d = nc.dram_tensor("x", [BPC, 128, CB, H + 2, W + 2], bf16,
                         kind="ExternalInput").ap()
    w_d = nc.dram_tensor("w", [128, CB, KS, KS, QB, 128], bf16,
                         kind="ExternalInput").ap()
    b_d = nc.dram_tensor("bias", [128, QB], f32, kind="ExternalInput").ap()
    o_d = nc.dram_tensor("out", [BPC, QB, 128, H, W], f32, kind="ExternalOutput").ap()

    with TileContext(nc) as tc:
        with tc.tile_pool(name="const", bufs=1) as cpool, \
             tc.tile_pool(name="xpad", bufs=3) as xpool, \
             tc.tile_pool(name="ot", bufs=4) as opool, \
             tc.tile_pool(name="ps", bufs=4, space="PSUM") as ppool:
            # image 0 load first (critical path), chunked over rows
            xps = []
            row_chunks = [(0, 18), (18, 34), (34, 50), (50, 66)]
            xp0 = xpool.tile([128, CB, H + 2, W + 2], bf16, tag="xp")
            for (r0, r1) in row_chunks:
                nc.sync.dma_start(out=xp0[:, :, r0:r1, :],
                                  in_=x_d[0, :, :, r0:r1, :])
            xps.append(xp0)

            w_sb = cpool.tile([128, CB, KS, KS, QB, 128], bf16)
            nc.sync.dma_start(out=w_sb[:], in_=w_d[:])
            bias_sb = cpool.tile([128, QB], f32)
            nc.sync.dma_start(out=bias_sb[:], in_=b_d[:])

            for b in range(BPC):
                if b < len(xps):
                    xp = xps[b]
                else:
                    xp = xpool.tile([128, CB, H + 2, W + 2], bf16, tag="xp")
                    nc.sync.dma_start(out=xp[:], in_=x_d[b])

                for cb in range(QB):
                    for rb in range(NRB):
                        y0 = rb * NROW
                        ps = ppool.tile([128, NROW * W], f32)
                        n_mm = CB * KS * KS
                        i = 0
                        for ib in range(CB):
                            for kh in range(KS):
                                for kw in range(KS):
                                    nc.tensor.matmul(
                                        ps[:],
                                        lhsT=w_sb[:, ib, kh, kw, cb, :],
                                        rhs=xp[:, ib, y0 + kh:y0 + kh + NROW,
                                               kw:kw + W],
                                        start=(i == 0),
                                        stop=(i == n_mm - 1),
                                    )
                                    i += 1
                        ot = opool.tile([128, NROW * W], f32)
                        nc.vector.tensor_scalar_add(ot[:], ps[:],
                                                    bias_sb[:, cb:cb + 1])
                        nc.sync.dma_start(out=o_d[b, cb, :, y0:y0 + NROW, :],
                                          in_=ot[:])
    nc.compile()
    return nc


def build_wino():
    import concourse.mybir as mybir
    from concourse.tile import TileContext

    bf16 = mybir.dt.bfloat16
    f32 = mybir.dt.float32
    NROW = 16                    # output rows per group (N = 16*32 = 512)
    NRB = H // NROW              # 4 groups per (img, co-blk)
    NT = W // 2                  # 32 Winograd tiles per row
    HP = H + 2                   # padded rows

    nc = _make_nc()
    # 4 column planes per (img, blk): xe, xo, xe1, xo1 -- each [66 rows, 32]
    x_d = nc.dram_tensor("x", [BPC, 128, CB, 4, HP, NT], bf16,
                         kind="ExternalInput").ap()
    # Winograd-transformed weights U[p], p=0..3
    w_d = nc.dram_tensor("w", [128, QB, CB, KS, 4, 128], bf16,
                         kind="ExternalInput").ap()
    b_d = nc.dram_tensor("bias", [128, QB], f32, kind="ExternalInput").ap()
    o_d = nc.dram_tensor("out", [BPC, QB, 128, H, W], f32, kind="ExternalOutput").ap()

    def transform_ops(nc, v_sb, xeo, ranges):
        # planes: 0=xe, 1=xo, 2=xe1, 3=xo1. One thunk per DVE op so the
        # caller can meter them out between matmul groups.
        ops = []
        for (r0, r1) in ranges:
            rs = slice(r0, r1)
            for ib in range(CB):
                xe = xeo[:, ib, 0, rs]
                xo = xeo[:, ib, 1, rs]
                xe1 = xeo[:, ib, 2, rs]
                xo1 = xeo[:, ib, 3, rs]
                ops += [
                    lambda o=v_sb[:, ib, 0, rs], a=xe, b=xe1:
                        nc.vector.tensor_sub(o, a, b),          # V0=d0-d2
                    lambda o=v_sb[:, ib, 1, rs], a=xo, b=xe1:
                        nc.vector.tensor_add(o, a, b),          # V1=d1+d2
                    lambda o=v_sb[:, ib, 2, rs], a=xe1, b=xo:
                        nc.vector.tensor_sub(o, a, b),          # V2=d2-d1
                    lambda o=v_sb[:, ib, 3, rs], a=xo, b=xo1:
                        nc.vector.tensor_sub(o, a, b),          # V3=d1-d3
                ]
        return ops

    def transform(nc, v_sb, xeo, r0, r1):
        for op in transform_ops(nc, v_sb, xeo, [(r0, r1)]):
            op()

    with TileContext(nc) as tc:
        with tc.tile_pool(name="const", bufs=1) as cpool, \
             tc.tile_pool(name="xeo", bufs=2) as xpool, \
             tc.tile_pool(name="vv", bufs=2) as vpool, \
             tc.tile_pool(name="ot", bufs=3) as opool, \
             tc.tile_pool(name="ps", bufs=2, space="PSUM") as ppool:
            # image 0: DMA + transform in row chunks to shorten the
            # critical path to the first matmul group
            chunks = [(0, 18), (18, 34), (34, 50), (50, 66)]
            # weights first: cb=0 half gates the very first matmul group
            w_sb = cpool.tile([128, QB, CB, KS, 4, 128], bf16)
            nc.sync.dma_start(out=w_sb[:, 0], in_=w_d[:, 0])
            bias_sb = cpool.tile([128, QB], f32)
            nc.sync.dma_start(out=bias_sb[:], in_=b_d[:])

            # image 0: fine-grained chunks up front to minimize the
            # DMA->transform->first-matmul critical path
            chunks0 = [(0, 9), (9, 18), (18, 34), (34, 50), (50, 66)]
            xeo0 = xpool.tile([128, CB, 4, HP, NT], bf16, tag="xeo")
            v0 = vpool.tile([128, CB, 4, HP, NT], bf16, tag="vv")
            r0, r1 = chunks0[0]
            nc.sync.dma_start(out=xeo0[:, :, :, r0:r1, :],
                              in_=x_d[0, :, :, :, r0:r1, :])
            nc.sync.dma_start(out=w_sb[:, 1], in_=w_d[:, 1])
            for (r0, r1) in chunks0[1:]:
                nc.sync.dma_start(out=xeo0[:, :, :, r0:r1, :],
                                  in_=x_d[0, :, :, :, r0:r1, :])
            for (r0, r1) in chunks0[:2]:
                transform(nc, v0, xeo0, r0, r1)

            vs = [v0]
            xeo = xeo0
            for b in range(BPC):
                v_sb = vs[b]
                # prefetch next image's planes; transform ops are metered
                # out between groups (~5/group) so DVE stays just under PE
                pend = transform_ops(nc, v_sb, xeo, [(18, 34), (34, 50),
                                                     (50, 66)]) \
                    if b == 0 else pend_next
                if b + 1 < BPC:
                    xeo_n = xpool.tile([128, CB, 4, HP, NT], bf16, tag="xeo")
                    nc.sync.dma_start(out=xeo_n[:], in_=x_d[b + 1])
                    v_n = vpool.tile([128, CB, 4, HP, NT], bf16, tag="vv")
                    vs.append(v_n)
                    pend += transform_ops(nc, v_n, xeo_n, [(0, 9), (9, 18)])
                    pend_next = transform_ops(nc, v_n, xeo_n,
                                              [(18, 34), (34, 50), (50, 66)])
                    xeo = xeo_n

                def emit_group(cb, y0, nrow):
                    # split the 4 m-slots over two PSUM tiles: A holds m0/m3
                    # (released early by the ACT seeds), B holds m1/m2
                    # (released by the DVE adds) -- the next-next group's
                    # first 12 matmuls only need an A tile, so PE keeps
                    # running while DVE drains B
                    psa = ppool.tile([128, 2, NROW * NT], f32, tag="psa")
                    psb = ppool.tile([128, 2, NROW * NT], f32, tag="psb")
                    slot = {0: psa[:, 0], 3: psa[:, 1],
                            1: psb[:, 0], 2: psb[:, 1]}
                    for p in (0, 3, 1, 2):
                        i = 0
                        for ib in range(CB):
                            for kh in range(KS):
                                nc.tensor.matmul(
                                    slot[p][:, :nrow * NT],
                                    lhsT=w_sb[:, cb, ib, kh, p, :],
                                    rhs=v_sb[:, ib, p, y0 + kh:y0 + kh + nrow, :],
                                    start=(i == 0),
                                    stop=(i == CB * KS - 1),
                                )
                                i += 1
                    ot = opool.tile([128, NROW, W], f32, tag="ot")
                    ot = ot[:, :nrow]
                    ev = ot[:, :, 0:W:2]
                    od = ot[:, :, 1:W:2]
                    m = [slot[p][:, :nrow * NT]
                         .rearrange("p (r t) -> p r t", t=NT) for p in range(4)]
                    # Y_even = m0+m1+m2+bias ; Y_odd = m1-m2-m3+bias
                    nc.scalar.activation(
                        ev, m[0], mybir.ActivationFunctionType.Identity,
                        bias=bias_sb[:, cb:cb + 1], scale=1.0)
                    nc.scalar.activation(
                        od, m[3], mybir.ActivationFunctionType.Identity,
                        bias=bias_sb[:, cb:cb + 1], scale=-1.0)
                    nc.vector.tensor_add(ev, ev, m[1])
                    nc.vector.tensor_add(ev, ev, m[2])
                    nc.vector.tensor_add(od, od, m[1])
                    nc.vector.tensor_sub(od, od, m[2])
                    nc.sync.dma_start(out=o_d[b, cb, :, y0:y0 + nrow, :],
                                      in_=ot[:])

                first = (b == 0)
                last = (b == BPC - 1)
                # (rb, cb) order: V row-chunks are consumed progressively,
                # so metered transforms always land a group ahead of use
                for rb in range(NRB):
                    for cb in range(QB):
                        y0 = rb * NROW
                        if first and rb == 0 and cb == 0:
                            emit_group(cb, 0, NROW // 2)
                            for op in pend[:3]:
                                op()
                            del pend[:3]
                            emit_group(cb, NROW // 2, NROW // 2)
                        elif last and rb == NRB - 1 and cb == QB - 1:
                            emit_group(cb, y0, NROW // 2)
                            emit_group(cb, y0 + NROW // 2, NROW // 2)
                        else:
                            emit_group(cb, y0, NROW)
                        n = min(5, len(pend))
                        for op in pend[:n]:
                            op()
                        del pend[:n]
                for op in pend:
                    op()
    nc.compile()
    return nc


def prep_direct(x, weight, bias):
    import ml_dtypes
    xs = x.reshape(NCORES, BPC, CB, 128, H, W).transpose(0, 1, 3, 2, 4, 5)
    xs = np.pad(xs, [(0, 0), (0, 0), (0, 0), (0, 0), (1, 1), (1, 1)])
    xs = np.ascontiguousarray(xs).astype(ml_dtypes.bfloat16)
    # w[co, ci, kh, kw] -> [ci, ci_blk, kh, kw, co_blk, co]
    wp = weight.reshape(QB, 128, CB, 128, KS, KS).transpose(3, 2, 4, 5, 0, 1)
    wp = np.ascontiguousarray(wp).astype(ml_dtypes.bfloat16)
    bp = np.ascontiguousarray(bias.reshape(QB, 128).T).astype(np.float32)
    return [{"x": xs[c], "w": wp, "bias": bp} for c in range(NCORES)]


def prep_wino(x, weight, bias):
    import ml_dtypes
    NT = W // 2
    xs = x.reshape(NCORES, BPC, CB, 128, H, W).transpose(0, 1, 3, 2, 4, 5)
    xs = np.pad(xs, [(0, 0)] * 4 + [(1, 1), (1, 1)])  # [NC,BPC,128,CB,66,66]
    xe = xs[..., 0::2]                                 # cols 0,2,..,64  (33)
    xo = xs[..., 1::2]                                 # cols 1,3,..,65  (33)
    planes = np.stack([xe[..., 0:NT], xo[..., 0:NT],
                       xe[..., 1:NT + 1], xo[..., 1:NT + 1]], axis=4)
    # [NC, BPC, 128, CB, 4, 66, NT]
    xp = np.ascontiguousarray(planes).astype(ml_dtypes.bfloat16)
    # U[p] from g=w[..,kh,:]: U0=g0, U1=(g0+g1+g2)/2, U2=(g0-g1+g2)/2, U3=g2
    g = weight.astype(np.float64)
    u = np.stack([g[..., 0],
                  (g[..., 0] + g[..., 1] + g[..., 2]) * 0.5,
                  (g[..., 0] - g[..., 1] + g[..., 2]) * 0.5,
                  g[..., 2]], axis=-1)                 # [co, ci, kh, 4]
    up = u.reshape(QB, 128, CB, 128, KS, 4).transpose(3, 0, 2, 4, 5, 1)
    up = np.ascontiguousarray(up).astype(ml_dtypes.bfloat16)
    bp = np.ascontiguousarray(bias.reshape(QB, 128).T).astype(np.float32)
    return [{"x": xp[c], "w": up, "bias": bp} for c in range(NCORES)]


def build_program():
    return build_wino() if ALGO == "wino" else build_direct()


def prep_inputs(x, weight, bias):
    return (prep_wino if ALGO == "wino" else prep_direct)(x, weight, bias)


def kernel(x, weight, bias):
    global _prog
    from concourse.bass_utils import run_bass_kernel_spmd

    if _prog is None:
        _prog = build_program()
    in_maps = prep_inputs(np.asarray(x, dtype=np.float32),
                          np.asarray(weight, dtype=np.float32),
                          np.asarray(bias, dtype=np.float32))
    res = run_bass_kernel_spmd(_prog, in_maps, list(range(NCORES)))
    outs = [r["out"].reshape(BPC, COUT, H, W) for r in res.results]
    return np.concatenate(outs, axis=0).astype(np.float32)


# revision 25
# speedup vs baseline: 1.0606x; 1.0031x over previous
"""Conv2d 3x3 (B=32, 256->256 ch, 64x64, pad 1) on 8 trn2 NeuronCores.

Data-parallel over batch: 4 images per core, weight/bias replicated.

Two algorithms:
- 'direct': implicit GEMM, 9 taps x 2 ci-blocks of shifted matmuls
  accumulating in PSUM (bf16 inputs, fp32 accumulation).
- 'wino': 1-D Winograd F(2,3) along W. PE work drops 1.5x (6 effective
  taps instead of 9). Host deinterleaves the padded image into 4
  aligned column planes (xe, xo, xe1, xo1) so the DVE input transform
  runs in 2x packed-bf16 mode; the 4 Winograd m-slots accumulate in 4
  PSUM banks per group; the inverse transform is 2 ACT ops (m0+bias /
  -m3+bias, reading PSUM) + 4 in-place DVE adds.
"""

import numpy as np

B, CIN, COUT, H, W, KS = 32, 256, 256, 64, 64, 3
NCORES = 8
BPC = B // NCORES            # images per core
CB = CIN // 128              # input-channel blocks
QB = COUT // 128             # output-channel blocks

ALGO = "wino"

_prog = None


def _make_nc():
    from concourse import bacc

    return bacc.Bacc("TRN2", target_bir_lowering=False, debug=False)


def build_direct():
    import concourse.mybir as mybir
    from concourse.tile import TileContext

    bf16 = mybir.dt.bfloat16
    f32 = mybir.dt.float32
    NROW = 8                     # output rows per matmul group (N = 512)
    NRB = H // NROW

    nc = _make_nc()
    x_d = nc.dram_tensor("x", [BPC, 128, CB, H + 2, W + 2], bf16,
                         kind="ExternalInput").ap()
    w_d = nc.dram_tensor("w", [128, CB, KS, KS, QB, 128], bf16,
                         kind="ExternalInput").ap()
    b_d = nc.dram_tensor("bias", [128, QB], f32, kind="ExternalInput").ap()
    o_d = nc.dram_tensor("out", [BPC, QB, 128, H, W], f32, kind="ExternalOutput").ap()

    with TileContext(nc) as tc:
        with tc.tile_pool(name="const", bufs=1) as cpool, \
             tc.tile_pool(name="xpad", bufs=3) as xpool, \
             tc.tile_pool(name="ot", bufs=4) as opool, \
             tc.tile_pool(name="ps", bufs=4, space="PSUM") as ppool:
            # image 0 load first (critical path), chunked over rows
            xps = []
            row_chunks = [(0, 18), (18, 34), (34, 50), (50, 66)]
            xp0 = xpool.tile([128, CB, H + 2, W + 2], bf16, tag="xp")
            for (r0, r1) in row_chunks:
                nc.sync.dma_start(out=xp0[:, :, r0:r1, :],
                                  in_=x_d[0, :, :, r0:r1, :])
            xps.append(xp0)

            w_sb = cpool.tile([128, CB, KS, KS, QB, 128], bf16)
            nc.sync.dma_start(out=w_sb[:], in_=w_d[:])
            bias_sb = cpool.tile([128, QB], f32)
            nc.sync.dma_start(out=bias_sb[:], in_=b_d[:])

            for b in range(BPC):
                if b < len(xps):
                    xp = xps[b]
                else:
                    xp = xpool.tile([128, CB, H + 2, W + 2], bf16, tag="xp")
                    nc.sync.dma_start(out=xp[:], in_=x_d[b])

                for cb in range(QB):
                    for rb in range(NRB):
                        y0 = rb * NROW
                        ps = ppool.tile([128, NROW * W], f32)
                        n_mm = CB * KS * KS
                        i = 0
                        for ib in range(CB):
                            for kh in range(KS):
                                for kw in range(KS):
                                    nc.tensor.matmul(
                                        ps[:],
                                        lhsT=w_sb[:, ib, kh, kw, cb, :],
                                        rhs=xp[:, ib, y0 + kh:y0 + kh + NROW,
                                               kw:kw + W],
                                        start=(i == 0),
                                        stop=(i == n_mm - 1),
                                    )
                                    i += 1
                        ot = opool.tile([128, NROW * W], f32)
                        nc.vector.tensor_scalar_add(ot[:], ps[:],
                                                    bias_sb[:, cb:cb + 1])
                        nc.sync.dma_start(out=o_d[b, cb, :, y0:y0 + NROW, :],
                                          in_=ot[:])
    nc.compile()
    return nc


def build_wino():
    import concourse.mybir as mybir
    from concourse.tile import TileContext

    bf16 = mybir.dt.bfloat16
    f32 = mybir.dt.float32
    NROW = 16                    # output rows per group (N = 16*32 = 512)
    NRB = H // NROW              # 4 groups per (img, co-blk)
    NT = W // 2                  # 32 Winograd tiles per row
    HP = H + 2                   # padded rows

    nc = _make_nc()
    # 4 column planes per (img, blk): xe, xo, xe1, xo1 -- each [66 rows, 32]
    x_d = nc.dram_tensor("x", [BPC, 128, CB, 4, HP, NT], bf16,
                         kind="ExternalInput").ap()
    # Winograd-transformed weights U[p], p=0..3
    w_d = nc.dram_tensor("w", [128, QB, CB, KS, 4, 128], bf16,
                         kind="ExternalInput").ap()
    b_d = nc.dram_tensor("bias", [128, QB], f32, kind="ExternalInput").ap()
    o_d = nc.dram_tensor("out", [BPC, QB, 128, H, W], f32, kind="ExternalOutput").ap()

    def transform_ops(nc, v_sb, xeo, ranges):
        # planes: 0=xe, 1=xo, 2=xe1, 3=xo1. One thunk per DVE op so the
        # caller can meter them out between matmul groups.
        ops = []
        for (r0, r1) in ranges:
            rs = slice(r0, r1)
            for ib in range(CB):
                xe = xeo[:, ib, 0, rs]
                xo = xeo[:, ib, 1, rs]
                xe1 = xeo[:, ib, 2, rs]
                xo1 = xeo[:, ib, 3, rs]
                ops += [
                    lambda o=v_sb[:, ib, 0, rs], a=xe, b=xe1:
                        nc.vector.tensor_sub(o, a, b),          # V0=d0-d2
                    lambda o=v_sb[:, ib, 1, rs], a=xo, b=xe1:
                        nc.vector.tensor_add(o, a, b),          # V1=d1+d2
                    lambda o=v_sb[:, ib, 2, rs], a=xe1, b=xo:
                        nc.vector.tensor_sub(o, a, b),          # V2=d2-d1
                    lambda o=v_sb[:, ib, 3, rs], a=xo, b=xo1:
                        nc.vector.tensor_sub(o, a, b),          # V3=d1-d3
                ]
        return ops

    def transform(nc, v_sb, xeo, r0, r1):
        for op in transform_ops(nc, v_sb, xeo, [(r0, r1)]):
            op()

    with TileContext(nc) as tc:
        with tc.tile_pool(name="const", bufs=1) as cpool, \
             tc.tile_pool(name="xeo", bufs=2) as xpool, \
             tc.tile_pool(name="vv", bufs=2) as vpool, \
             tc.tile_pool(name="ot", bufs=3) as opool, \
             tc.tile_pool(name="ps", bufs=2, space="PSUM") as ppool:
            # image 0: DMA + transform in row chunks to shorten the
            # critical path to the first matmul group
            chunks = [(0, 18), (18, 34), (34, 50), (50, 66)]
            # weights first: cb=0 half gates the very first matmul group
            w_sb = cpool.tile([128, QB, CB, KS, 4, 128], bf16)
            nc.sync.dma_start(out=w_sb[:, 0, 0], in_=w_d[:, 0, 0])
            bias_sb = cpool.tile([128, QB], f32)
            nc.sync.dma_start(out=bias_sb[:], in_=b_d[:])

            # image 0: fine-grained chunks up front to minimize the
            # DMA->transform->first-matmul critical path
            chunks0 = [(0, 9), (9, 18), (18, 34), (34, 50), (50, 66)]
            xeo0 = xpool.tile([128, CB, 4, HP, NT], bf16, tag="xeo")
            v0 = vpool.tile([128, CB, 4, HP, NT], bf16, tag="vv")
            r0, r1 = chunks0[0]
            nc.sync.dma_start(out=xeo0[:, :, :, r0:r1, :],
                              in_=x_d[0, :, :, :, r0:r1, :])
            nc.sync.dma_start(out=w_sb[:, 0, 1], in_=w_d[:, 0, 1])
            for ib in range(CB):
                nc.sync.dma_start(out=w_sb[:, 1, ib], in_=w_d[:, 1, ib])
            for (r0, r1) in chunks0[1:]:
                nc.sync.dma_start(out=xeo0[:, :, :, r0:r1, :],
                                  in_=x_d[0, :, :, :, r0:r1, :])
            for (r0, r1) in chunks0[:2]:
                transform(nc, v0, xeo0, r0, r1)

            vs = [v0]
            xeo = xeo0
            for b in range(BPC):
                v_sb = vs[b]
                # prefetch next image's planes; transform ops are metered
                # out between groups (~5/group) so DVE stays just under PE
                pend = transform_ops(nc, v_sb, xeo, [(18, 34), (34, 50),
                                                     (50, 66)]) \
                    if b == 0 else pend_next
                if b + 1 < BPC:
                    xeo_n = xpool.tile([128, CB, 4, HP, NT], bf16, tag="xeo")
                    v_n = vpool.tile([128, CB, 4, HP, NT], bf16, tag="vv")
                    vs.append(v_n)
                    pend += transform_ops(nc, v_n, xeo_n, [(0, 9), (9, 18)])
                    pend_next = transform_ops(nc, v_n, xeo_n,
                                              [(18, 34), (34, 50), (50, 66)])
                    xeo = xeo_n

                def emit_group(cb, y0, nrow):
                    # split the 4 m-slots over two PSUM tiles: A holds m0/m3
                    # (released early by the ACT seeds), B holds m1/m2
                    # (released by the DVE adds) -- the next-next group's
                    # first 12 matmuls only need an A tile, so PE keeps
                    # running while DVE drains B
                    psa = ppool.tile([128, 2, NROW * NT], f32, tag="psa")
                    psb = ppool.tile([128, 2, NROW * NT], f32, tag="psb")
                    slot = {0: psa[:, 0], 3: psa[:, 1],
                            1: psb[:, 0], 2: psb[:, 1]}
                    for p in (0, 3, 1, 2):
                        i = 0
                        for ib in range(CB):
                            for kh in range(KS):
                                nc.tensor.matmul(
                                    slot[p][:, :nrow * NT],
                                    lhsT=w_sb[:, cb, ib, kh, p, :],
                                    rhs=v_sb[:, ib, p, y0 + kh:y0 + kh + nrow, :],
                                    start=(i == 0),
                                    stop=(i == CB * KS - 1),
                                )
                                i += 1
                    ot = opool.tile([128, NROW, W], f32, tag="ot")
                    ot = ot[:, :nrow]
                    ev = ot[:, :, 0:W:2]
                    od = ot[:, :, 1:W:2]
                    m = [slot[p][:, :nrow * NT]
                         .rearrange("p (r t) -> p r t", t=NT) for p in range(4)]
                    # Y_even = m0+m1+m2+bias ; Y_odd = m1-m2-m3+bias
                    nc.scalar.activation(
                        ev, m[0], mybir.ActivationFunctionType.Identity,
                        bias=bias_sb[:, cb:cb + 1], scale=1.0)
                    nc.scalar.activation(
                        od, m[3], mybir.ActivationFunctionType.Identity,
                        bias=bias_sb[:, cb:cb + 1], scale=-1.0)
                    nc.vector.tensor_add(ev, ev, m[1])
                    nc.vector.tensor_add(ev, ev, m[2])
                    nc.vector.tensor_add(od, od, m[1])
                    nc.vector.tensor_sub(od, od, m[2])
                    nc.sync.dma_start(out=o_d[b, cb, :, y0:y0 + nrow, :],
                                      in_=ot[:])

                first = (b == 0)
                last = (b == BPC - 1)
                # (rb, cb) order: V row-chunks are consumed progressively,
                # so metered transforms always land a group ahead of use
                for rb in range(NRB):
                    for cb in range(QB):
                        y0 = rb * NROW
                        if first and rb == 0 and cb == 0:
                            emit_group(cb, 0, NROW // 2)
                            for op in pend[:3]:
                                op()
                            del pend[:3]
                            emit_group(cb, NROW // 2, NROW // 2)
                        elif last and rb == NRB - 1 and cb == QB - 1:
                            emit_group(cb, y0, NROW // 2)
                            emit_group(cb, y0 + NROW // 2, NROW // 2)
                        else:
                            emit_group(cb, y0, NROW)
                        if b + 1 < BPC and rb == 1 and cb == 0:
                            nc.sync.dma_start(out=xeo_n[:], in_=x_d[b + 1])
                        n = min(5, len(pend))
                        for op in pend[:n]:
                            op()
                        del pend[:n]
                for op in pend:
                    op()
    nc.compile()
    return nc


def prep_direct(x, weight, bias):
    import ml_dtypes
    xs = x.reshape(NCORES, BPC, CB, 128, H, W).transpose(0, 1, 3, 2, 4, 5)
    xs = np.pad(xs, [(0, 0), (0, 0), (0, 0), (0, 0), (1, 1), (1, 1)])
    xs = np.ascontiguousarray(xs).astype(ml_dtypes.bfloat16)
    # w[co, ci, kh, kw] -> [ci, ci_blk, kh, kw, co_blk, co]
    wp = weight.reshape(QB, 128, CB, 128, KS, KS).transpose(3, 2, 4, 5, 0, 1)
    wp = np.ascontiguousarray(wp).astype(ml_dtypes.bfloat16)
    bp = np.ascontiguousarray(bias.reshape(QB, 128).T).astype(np.float32)
    return [{"x": xs[c], "w": wp, "bias": bp} for c in range(NCORES)]


def prep_wino(x, weight, bias):
    import ml_dtypes
    NT = W // 2
    xs = x.reshape(NCORES, BPC, CB, 128, H, W).transpose(0, 1, 3, 2, 4, 5)
    xs = np.pad(xs, [(0, 0)] * 4 + [(1, 1), (1, 1)])  # [NC,BPC,128,CB,66,66]
    xe = xs[..., 0::2]                                 # cols 0,2,..,64  (33)
    xo = xs[..., 1::2]                                 # cols 1,3,..,65  (33)
    planes = np.stack([xe[..., 0:NT], xo[..., 0:NT],
                       xe[..., 1:NT + 1], xo[..., 1:NT + 1]], axis=4)
    # [NC, BPC, 128, CB, 4, 66, NT]
    xp = np.ascontiguousarray(planes).astype(ml_dtypes.bfloat16)
    # U[p] from g=w[..,kh,:]: U0=g0, U1=(g0+g1+g2)/2, U2=(g0-g1+g2)/2, U3=g2
    g = weight.astype(np.float64)
    u = np.stack([g[..., 0],
                  (g[..., 0] + g[..., 1] + g[..., 2]) * 0.5,
                  (g[..., 0] - g[..., 1] + g[..., 2]) * 0.5,
                  g[..., 2]], axis=-1)                 # [co, ci, kh, 4]
    up = u.reshape(QB, 128, CB, 128, KS, 4).transpose(3, 0, 2, 4, 5, 1)
    up = np.ascontiguousarray(up).astype(ml_dtypes.bfloat16)
    bp = np.ascontiguousarray(bias.reshape(QB, 128).T).astype(np.float32)
    return [{"x": xp[c], "w": up, "bias": bp} for c in range(NCORES)]


def build_program():
    return build_wino() if ALGO == "wino" else build_direct()


def prep_inputs(x, weight, bias):
    return (prep_wino if ALGO == "wino" else prep_direct)(x, weight, bias)


def kernel(x, weight, bias):
    global _prog
    from concourse.bass_utils import run_bass_kernel_spmd

    if _prog is None:
        _prog = build_program()
    in_maps = prep_inputs(np.asarray(x, dtype=np.float32),
                          np.asarray(weight, dtype=np.float32),
                          np.asarray(bias, dtype=np.float32))
    res = run_bass_kernel_spmd(_prog, in_maps, list(range(NCORES)))
    outs = [r["out"].reshape(BPC, COUT, H, W) for r in res.results]
    return np.concatenate(outs, axis=0).astype(np.float32)


# revision 26
# speedup vs baseline: 1.0788x; 1.0172x over previous
"""Conv2d 3x3 (B=32, 256->256 ch, 64x64, pad 1) on 8 trn2 NeuronCores.

Data-parallel over batch: 4 images per core, weight/bias replicated.

Two algorithms:
- 'direct': implicit GEMM, 9 taps x 2 ci-blocks of shifted matmuls
  accumulating in PSUM (bf16 inputs, fp32 accumulation).
- 'wino': 1-D Winograd F(2,3) along W. PE work drops 1.5x (6 effective
  taps instead of 9). Host deinterleaves the padded image into 4
  aligned column planes (xe, xo, xe1, xo1) so the DVE input transform
  runs in 2x packed-bf16 mode; the 4 Winograd m-slots accumulate in 4
  PSUM banks per group; the inverse transform is 2 ACT ops (m0+bias /
  -m3+bias, reading PSUM) + 4 in-place DVE adds.
"""

import numpy as np

B, CIN, COUT, H, W, KS = 32, 256, 256, 64, 64, 3
NCORES = 8
BPC = B // NCORES            # images per core
CB = CIN // 128              # input-channel blocks
QB = COUT // 128             # output-channel blocks

ALGO = "wino"

_prog = None


def _make_nc():
    from concourse import bacc

    return bacc.Bacc("TRN2", target_bir_lowering=False, debug=False)


def build_direct():
    import concourse.mybir as mybir
    from concourse.tile import TileContext

    bf16 = mybir.dt.bfloat16
    f32 = mybir.dt.float32
    NROW = 8                     # output rows per matmul group (N = 512)
    NRB = H // NROW

    nc = _make_nc()
    x_d = nc.dram_tensor("x", [BPC, 128, CB, H + 2, W + 2], bf16,
                         kind="ExternalInput").ap()
    w_d = nc.dram_tensor("w", [128, CB, KS, KS, QB, 128], bf16,
                         kind="ExternalInput").ap()
    b_d = nc.dram_tensor("bias", [128, QB], f32, kind="ExternalInput").ap()
    o_d = nc.dram_tensor("out", [BPC, QB, 128, H, W], f32, kind="ExternalOutput").ap()

    with TileContext(nc) as tc:
        with tc.tile_pool(name="const", bufs=1) as cpool, \
             tc.tile_pool(name="xpad", bufs=3) as xpool, \
             tc.tile_pool(name="ot", bufs=4) as opool, \
             tc.tile_pool(name="ps", bufs=4, space="PSUM") as ppool:
            # image 0 load first (critical path), chunked over rows
            xps = []
            row_chunks = [(0, 18), (18, 34), (34, 50), (50, 66)]
            xp0 = xpool.tile([128, CB, H + 2, W + 2], bf16, tag="xp")
            for (r0, r1) in row_chunks:
                nc.sync.dma_start(out=xp0[:, :, r0:r1, :],
                                  in_=x_d[0, :, :, r0:r1, :])
            xps.append(xp0)

            w_sb = cpool.tile([128, CB, KS, KS, QB, 128], bf16)
            nc.sync.dma_start(out=w_sb[:], in_=w_d[:])
            bias_sb = cpool.tile([128, QB], f32)
            nc.sync.dma_start(out=bias_sb[:], in_=b_d[:])

            for b in range(BPC):
                if b < len(xps):
                    xp = xps[b]
                else:
                    xp = xpool.tile([128, CB, H + 2, W + 2], bf16, tag="xp")
                    nc.sync.dma_start(out=xp[:], in_=x_d[b])

                for cb in range(QB):
                    for rb in range(NRB):
                        y0 = rb * NROW
                        ps = ppool.tile([128, NROW * W], f32)
                        n_mm = CB * KS * KS
                        i = 0
                        for ib in range(CB):
                            for kh in range(KS):
                                for kw in range(KS):
                                    nc.tensor.matmul(
                                        ps[:],
                                        lhsT=w_sb[:, ib, kh, kw, cb, :],
                                        rhs=xp[:, ib, y0 + kh:y0 + kh + NROW,
                                               kw:kw + W],
                                        start=(i == 0),
                                        stop=(i == n_mm - 1),
                                    )
                                    i += 1
                        ot = opool.tile([128, NROW * W], f32)
                        nc.vector.tensor_scalar_add(ot[:], ps[:],
                                                    bias_sb[:, cb:cb + 1])
                        nc.sync.dma_start(out=o_d[b, cb, :, y0:y0 + NROW, :],
                                          in_=ot[:])
    nc.compile()
    return nc


def build_wino():
    import concourse.mybir as mybir
    from concourse.tile import TileContext

    bf16 = mybir.dt.bfloat16
    f32 = mybir.dt.float32
    NROW = 16                    # output rows per group (N = 16*32 = 512)
    NRB = H // NROW              # 4 groups per (img, co-blk)
    NT = W // 2                  # 32 Winograd tiles per row
    HP = H + 2                   # padded rows

    nc = _make_nc()
    # 4 column planes per (img, blk): xe, xo, xe1, xo1 -- each [66 rows, 32]
    x_d = nc.dram_tensor("x", [BPC, 128, CB, 4, HP, NT], bf16,
                         kind="ExternalInput").ap()
    # Winograd-transformed weights U[p], p=0..3
    w_d = nc.dram_tensor("w", [128, QB, CB, KS, 4, 128], bf16,
                         kind="ExternalInput").ap()
    b_d = nc.dram_tensor("bias", [128, QB], f32, kind="ExternalInput").ap()
    o_d = nc.dram_tensor("out", [BPC, QB, 128, H, W], f32, kind="ExternalOutput").ap()

    def transform_ops(nc, v_sb, xeo, ranges):
        # planes: 0=xe, 1=xo, 2=xe1, 3=xo1. One thunk per DVE op so the
        # caller can meter them out between matmul groups.
        ops = []
        for (r0, r1) in ranges:
            rs = slice(r0, r1)
            for ib in range(CB):
                xe = xeo[:, ib, 0, rs]
                xo = xeo[:, ib, 1, rs]
                xe1 = xeo[:, ib, 2, rs]
                xo1 = xeo[:, ib, 3, rs]
                ops += [
                    lambda o=v_sb[:, ib, 0, rs], a=xe, b=xe1:
                        nc.vector.tensor_sub(o, a, b),          # V0=d0-d2
                    lambda o=v_sb[:, ib, 1, rs], a=xo, b=xe1:
                        nc.vector.tensor_add(o, a, b),          # V1=d1+d2
                    lambda o=v_sb[:, ib, 2, rs], a=xe1, b=xo:
                        nc.vector.tensor_sub(o, a, b),          # V2=d2-d1
                    lambda o=v_sb[:, ib, 3, rs], a=xo, b=xo1:
                        nc.vector.tensor_sub(o, a, b),          # V3=d1-d3
                ]
        return ops

    def transform(nc, v_sb, xeo, r0, r1):
        # emit in MM pos order (V0, V3, V1, V2 across both blocks) so the
        # first matmuls unblock after 2 ops instead of 8
        ops = transform_ops(nc, v_sb, xeo, [(r0, r1)])
        for i in (0, 4, 3, 7, 1, 5, 2, 6):
            ops[i]()

    with TileContext(nc) as tc:
        with tc.tile_pool(name="const", bufs=1) as cpool, \
             tc.tile_pool(name="xeo", bufs=2) as xpool, \
             tc.tile_pool(name="vv", bufs=2) as vpool, \
             tc.tile_pool(name="ot", bufs=3) as opool, \
             tc.tile_pool(name="ps", bufs=2, space="PSUM") as ppool:
            # image 0: DMA + transform in row chunks to shorten the
            # critical path to the first matmul group
            chunks = [(0, 18), (18, 34), (34, 50), (50, 66)]
            # weights first: cb=0 half gates the very first matmul group
            w_sb = cpool.tile([128, QB, CB, KS, 4, 128], bf16)
            nc.sync.dma_start(out=w_sb[:, 0, 0], in_=w_d[:, 0, 0])
            bias_sb = cpool.tile([128, QB], f32)
            nc.sync.dma_start(out=bias_sb[:], in_=b_d[:])

            # image 0: fine-grained chunks up front to minimize the
            # DMA->transform->first-matmul critical path
            chunks0 = [(0, 9), (9, 18), (18, 34), (34, 50), (50, 66)]
            xeo0 = xpool.tile([128, CB, 4, HP, NT], bf16, tag="xeo")
            v0 = vpool.tile([128, CB, 4, HP, NT], bf16, tag="vv")
            r0, r1 = chunks0[0]
            nc.sync.dma_start(out=xeo0[:, :, :, r0:r1, :],
                              in_=x_d[0, :, :, :, r0:r1, :])
            nc.sync.dma_start(out=w_sb[:, 0, 1], in_=w_d[:, 0, 1])
            for ib in range(CB):
                nc.sync.dma_start(out=w_sb[:, 1, ib], in_=w_d[:, 1, ib])
            for (r0, r1) in chunks0[1:3]:
                nc.sync.dma_start(out=xeo0[:, :, :, r0:r1, :],
                                  in_=x_d[0, :, :, :, r0:r1, :])
            for (r0, r1) in chunks0[:2]:
                transform(nc, v0, xeo0, r0, r1)

            vs = [v0]
            xeo = xeo0
            for b in range(BPC):
                v_sb = vs[b]
                # prefetch next image's planes; transform ops are metered
                # out between groups (~5/group) so DVE stays just under PE
                pend = transform_ops(nc, v_sb, xeo, [(18, 34), (34, 50),
                                                     (50, 66)]) \
                    if b == 0 else pend_next
                if b + 1 < BPC:
                    xeo_n = xpool.tile([128, CB, 4, HP, NT], bf16, tag="xeo")
                    v_n = vpool.tile([128, CB, 4, HP, NT], bf16, tag="vv")
                    vs.append(v_n)
                    pend += transform_ops(nc, v_n, xeo_n, [(0, 9), (9, 18)])
                    pend_next = transform_ops(nc, v_n, xeo_n,
                                              [(18, 34), (34, 50), (50, 66)])
                    xeo = xeo_n

                def emit_group(cb, y0, nrow):
                    # split the 4 m-slots over two PSUM tiles: A holds m0/m3
                    # (released early by the ACT seeds), B holds m1/m2
                    # (released by the DVE adds) -- the next-next group's
                    # first 12 matmuls only need an A tile, so PE keeps
                    # running while DVE drains B
                    psa = ppool.tile([128, 2, NROW * NT], f32, tag="psa")
                    psb = ppool.tile([128, 2, NROW * NT], f32, tag="psb")
                    slot = {0: psa[:, 0], 3: psa[:, 1],
                            1: psb[:, 0], 2: psb[:, 1]}
                    for p in (0, 3, 1, 2):
                        i = 0
                        for ib in range(CB):
                            for kh in range(KS):
                                nc.tensor.matmul(
                                    slot[p][:, :nrow * NT],
                                    lhsT=w_sb[:, cb, ib, kh, p, :],
                                    rhs=v_sb[:, ib, p, y0 + kh:y0 + kh + nrow, :],
                                    start=(i == 0),
                                    stop=(i == CB * KS - 1),
                                )
                                i += 1
                    ot = opool.tile([128, NROW, W], f32, tag="ot")
                    ot = ot[:, :nrow]
                    ev = ot[:, :, 0:W:2]
                    od = ot[:, :, 1:W:2]
                    m = [slot[p][:, :nrow * NT]
                         .rearrange("p (r t) -> p r t", t=NT) for p in range(4)]
                    # Y_even = m0+m1+m2+bias ; Y_odd = m1-m2-m3+bias
                    nc.scalar.activation(
                        ev, m[0], mybir.ActivationFunctionType.Identity,
                        bias=bias_sb[:, cb:cb + 1], scale=1.0)
                    nc.scalar.activation(
                        od, m[3], mybir.ActivationFunctionType.Identity,
                        bias=bias_sb[:, cb:cb + 1], scale=-1.0)
                    nc.vector.tensor_add(ev, ev, m[1])
                    nc.vector.tensor_add(ev, ev, m[2])
                    nc.vector.tensor_add(od, od, m[1])
                    nc.vector.tensor_sub(od, od, m[2])
                    nc.sync.dma_start(out=o_d[b, cb, :, y0:y0 + nrow, :],
                                      in_=ot[:])

                first = (b == 0)
                last = (b == BPC - 1)
                # (rb, cb) order: V row-chunks are consumed progressively,
                # so metered transforms always land a group ahead of use
                for rb in range(NRB):
                    for cb in range(QB):
                        y0 = rb * NROW
                        if first and rb == 0 and cb == 0:
                            emit_group(cb, 0, NROW // 2)
                            for op in pend[:3]:
                                op()
                            del pend[:3]
                            emit_group(cb, NROW // 2, NROW // 2)
                        elif last and rb == NRB - 1 and cb == QB - 1:
                            emit_group(cb, y0, NROW // 2)
                            emit_group(cb, y0 + NROW // 2, NROW // 2)
                        else:
                            emit_group(cb, y0, NROW)
                        if b == 0 and rb == 0:
                            r0, r1 = chunks0[3 + cb]
                            nc.sync.dma_start(out=xeo0[:, :, :, r0:r1, :],
                                              in_=x_d[0, :, :, :, r0:r1, :])
                        if b + 1 < BPC and rb == 1 and cb == 0:
                            nc.sync.dma_start(out=xeo_n[:], in_=x_d[b + 1])
                        n = min(5, len(pend))
                        for op in pend[:n]:
                            op()
                        del pend[:n]
                for op in pend:
                    op()
    nc.compile()
    return nc


def prep_direct(x, weight, bias):
    import ml_dtypes
    xs = x.reshape(NCORES, BPC, CB, 128, H, W).transpose(0, 1, 3, 2, 4, 5)
    xs = np.pad(xs, [(0, 0), (0, 0), (0, 0), (0, 0), (1, 1), (1, 1)])
    xs = np.ascontiguousarray(xs).astype(ml_dtypes.bfloat16)
    # w[co, ci, kh, kw] -> [ci, ci_blk, kh, kw, co_blk, co]
    wp = weight.reshape(QB, 128, CB, 128, KS, KS).transpose(3, 2, 4, 5, 0, 1)
    wp = np.ascontiguousarray(wp).astype(ml_dtypes.bfloat16)
    bp = np.ascontiguousarray(bias.reshape(QB, 128).T).astype(np.float32)
    return [{"x": xs[c], "w": wp, "bias": bp} for c in range(NCORES)]


def prep_wino(x, weight, bias):
    import ml_dtypes
    NT = W // 2
    xs = x.reshape(NCORES, BPC, CB, 128, H, W).transpose(0, 1, 3, 2, 4, 5)
    xs = np.pad(xs, [(0, 0)] * 4 + [(1, 1), (1, 1)])  # [NC,BPC,128,CB,66,66]
    xe = xs[..., 0::2]                                 # cols 0,2,..,64  (33)
    xo = xs[..., 1::2]                                 # cols 1,3,..,65  (33)
    planes = np.stack([xe[..., 0:NT], xo[..., 0:NT],
                       xe[..., 1:NT + 1], xo[..., 1:NT + 1]], axis=4)
    # [NC, BPC, 128, CB, 4, 66, NT]
    xp = np.ascontiguousarray(planes).astype(ml_dtypes.bfloat16)
    # U[p] from g=w[..,kh,:]: U0=g0, U1=(g0+g1+g2)/2, U2=(g0-g1+g2)/2, U3=g2
    g = weight.astype(np.float64)
    u = np.stack([g[..., 0],
                  (g[..., 0] + g[..., 1] + g[..., 2]) * 0.5,
                  (g[..., 0] - g[..., 1] + g[..., 2]) * 0.5,
                  g[..., 2]], axis=-1)                 # [co, ci, kh, 4]
    up = u.reshape(QB, 128, CB, 128, KS, 4).transpose(3, 0, 2, 4, 5, 1)
    up = np.ascontiguousarray(up).astype(ml_dtypes.bfloat16)
    bp = np.ascontiguousarray(bias.reshape(QB, 128).T).astype(np.float32)
    return [{"x": xp[c], "w": up, "bias": bp} for c in range(NCORES)]


def build_program():
    return build_wino() if ALGO == "wino" else build_direct()


def prep_inputs(x, weight, bias):
    return (prep_wino if ALGO == "wino" else prep_direct)(x, weight, bias)


def kernel(x, weight, bias):
    global _prog
    from concourse.bass_utils import run_bass_kernel_spmd

    if _prog is None:
        _prog = build_program()
    in_maps = prep_inputs(np.asarray(x, dtype=np.float32),
                          np.asarray(weight, dtype=np.float32),
                          np.asarray(bias, dtype=np.float32))
    res = run_bass_kernel_spmd(_prog, in_maps, list(range(NCORES)))
    outs = [r["out"].reshape(BPC, COUT, H, W) for r in res.results]
    return np.concatenate(outs, axis=0).astype(np.float32)


# revision 27
# speedup vs baseline: 1.1162x; 1.0347x over previous
"""Conv2d 3x3 (B=32, 256->256 ch, 64x64, pad 1) on 8 trn2 NeuronCores.

Data-parallel over batch: 4 images per core, weight/bias replicated.

Two algorithms:
- 'direct': implicit GEMM, 9 taps x 2 ci-blocks of shifted matmuls
  accumulating in PSUM (bf16 inputs, fp32 accumulation).
- 'wino': 1-D Winograd F(2,3) along W. PE work drops 1.5x (6 effective
  taps instead of 9). Host deinterleaves the padded image into 4
  aligned column planes (xe, xo, xe1, xo1) so the DVE input transform
  runs in 2x packed-bf16 mode; the 4 Winograd m-slots accumulate in 4
  PSUM banks per group; the inverse transform is 2 ACT ops (m0+bias /
  -m3+bias, reading PSUM) + 4 in-place DVE adds.
"""

import numpy as np

B, CIN, COUT, H, W, KS = 32, 256, 256, 64, 64, 3
NCORES = 8
BPC = B // NCORES            # images per core
CB = CIN // 128              # input-channel blocks
QB = COUT // 128             # output-channel blocks

ALGO = "wino"

_prog = None


def _make_nc():
    from concourse import bacc

    return bacc.Bacc("TRN2", target_bir_lowering=False, debug=False)


def build_direct():
    import concourse.mybir as mybir
    from concourse.tile import TileContext

    bf16 = mybir.dt.bfloat16
    f32 = mybir.dt.float32
    NROW = 8                     # output rows per matmul group (N = 512)
    NRB = H // NROW

    nc = _make_nc()
    x_d = nc.dram_tensor("x", [BPC, 128, CB, H + 2, W + 2], bf16,
                         kind="ExternalInput").ap()
    w_d = nc.dram_tensor("w", [128, CB, KS, KS, QB, 128], bf16,
                         kind="ExternalInput").ap()
    b_d = nc.dram_tensor("bias", [128, QB], f32, kind="ExternalInput").ap()
    o_d = nc.dram_tensor("out", [BPC, QB, 128, H, W], f32, kind="ExternalOutput").ap()

    with TileContext(nc) as tc:
        with tc.tile_pool(name="const", bufs=1) as cpool, \
             tc.tile_pool(name="xpad", bufs=3) as xpool, \
             tc.tile_pool(name="ot", bufs=4) as opool, \
             tc.tile_pool(name="ps", bufs=4, space="PSUM") as ppool:
            # image 0 load first (critical path), chunked over rows
            xps = []
            row_chunks = [(0, 18), (18, 34), (34, 50), (50, 66)]
            xp0 = xpool.tile([128, CB, H + 2, W + 2], bf16, tag="xp")
            for (r0, r1) in row_chunks:
                nc.sync.dma_start(out=xp0[:, :, r0:r1, :],
                                  in_=x_d[0, :, :, r0:r1, :])
            xps.append(xp0)

            w_sb = cpool.tile([128, CB, KS, KS, QB, 128], bf16)
            nc.sync.dma_start(out=w_sb[:], in_=w_d[:])
            bias_sb = cpool.tile([128, QB], f32)
            nc.sync.dma_start(out=bias_sb[:], in_=b_d[:])

            for b in range(BPC):
                if b < len(xps):
                    xp = xps[b]
                else:
                    xp = xpool.tile([128, CB, H + 2, W + 2], bf16, tag="xp")
                    nc.sync.dma_start(out=xp[:], in_=x_d[b])

                for cb in range(QB):
                    for rb in range(NRB):
                        y0 = rb * NROW
                        ps = ppool.tile([128, NROW * W], f32)
                        n_mm = CB * KS * KS
                        i = 0
                        for ib in range(CB):
                            for kh in range(KS):
                                for kw in range(KS):
                                    nc.tensor.matmul(
                                        ps[:],
                                        lhsT=w_sb[:, ib, kh, kw, cb, :],
                                        rhs=xp[:, ib, y0 + kh:y0 + kh + NROW,
                                               kw:kw + W],
                                        start=(i == 0),
                                        stop=(i == n_mm - 1),
                                    )
                                    i += 1
                        ot = opool.tile([128, NROW * W], f32)
                        nc.vector.tensor_scalar_add(ot[:], ps[:],
                                                    bias_sb[:, cb:cb + 1])
                        nc.sync.dma_start(out=o_d[b, cb, :, y0:y0 + NROW, :],
                                          in_=ot[:])
    nc.compile()
    return nc


def build_wino():
    import concourse.mybir as mybir
    from concourse.tile import TileContext

    bf16 = mybir.dt.bfloat16
    f32 = mybir.dt.float32
    NROW = 16                    # output rows per group (N = 16*32 = 512)
    NRB = H // NROW              # 4 groups per (img, co-blk)
    NT = W // 2                  # 32 Winograd tiles per row
    HP = H + 2                   # padded rows

    nc = _make_nc()
    # 4 column planes per (img, blk): xe, xo, xe1, xo1 -- each [66 rows, 32]
    x_d = nc.dram_tensor("x", [BPC, 128, CB, 4, HP, NT], bf16,
                         kind="ExternalInput").ap()
    # Winograd-transformed weights U[p], p=0..3
    w_d = nc.dram_tensor("w", [128, QB, CB, 4, KS, 128], bf16,
                         kind="ExternalInput").ap()
    b_d = nc.dram_tensor("bias", [128, QB], f32, kind="ExternalInput").ap()
    o_d = nc.dram_tensor("out", [BPC, QB, 128, H, W], f32, kind="ExternalOutput").ap()

    def transform_ops(nc, v_sb, xeo, ranges):
        # planes: 0=xe, 1=xo, 2=xe1, 3=xo1. One thunk per DVE op so the
        # caller can meter them out between matmul groups.
        ops = []
        for (r0, r1) in ranges:
            rs = slice(r0, r1)
            for ib in range(CB):
                xe = xeo[:, ib, 0, rs]
                xo = xeo[:, ib, 1, rs]
                xe1 = xeo[:, ib, 2, rs]
                xo1 = xeo[:, ib, 3, rs]
                ops += [
                    lambda o=v_sb[:, ib, 0, rs], a=xe, b=xe1:
                        nc.vector.tensor_sub(o, a, b),          # V0=d0-d2
                    lambda o=v_sb[:, ib, 1, rs], a=xo, b=xe1:
                        nc.vector.tensor_add(o, a, b),          # V1=d1+d2
                    lambda o=v_sb[:, ib, 2, rs], a=xe1, b=xo:
                        nc.vector.tensor_sub(o, a, b),          # V2=d2-d1
                    lambda o=v_sb[:, ib, 3, rs], a=xo, b=xo1:
                        nc.vector.tensor_sub(o, a, b),          # V3=d1-d3
                ]
        return ops

    def transform(nc, v_sb, xeo, r0, r1):
        # emit in MM pos order (V0, V3, V1, V2 across both blocks) so the
        # first matmuls unblock after 2 ops instead of 8
        ops = transform_ops(nc, v_sb, xeo, [(r0, r1)])
        for i in (0, 4, 3, 7, 1, 5, 2, 6):
            ops[i]()

    with TileContext(nc) as tc:
        with tc.tile_pool(name="const", bufs=1) as cpool, \
             tc.tile_pool(name="xeo", bufs=2) as xpool, \
             tc.tile_pool(name="vv", bufs=2) as vpool, \
             tc.tile_pool(name="ot", bufs=6) as opool, \
             tc.tile_pool(name="ps", bufs=2, space="PSUM") as ppool:
            # image 0: DMA + transform in row chunks to shorten the
            # critical path to the first matmul group
            chunks = [(0, 18), (18, 34), (34, 50), (50, 66)]
            # weights first: cb=0 half gates the very first matmul group
            w_sb = cpool.tile([128, QB, CB, 4, KS, 128], bf16)
            chunks0 = [(0, 9), (9, 18), (18, 34), (34, 50), (50, 66)]
            xeo0 = xpool.tile([128, CB, 4, HP, NT], bf16, tag="xeo")
            v0 = vpool.tile([128, CB, 4, HP, NT], bf16, tag="vv")
            # critical-path order: x rows 0-9 per block, then w(cb0) pos
            # slices in MM order -- each lands on its own DMA queue so the
            # first matmuls start ~10us in instead of ~19us
            for ib in range(CB):
                nc.sync.dma_start(out=xeo0[:, ib, :, 0:9, :],
                                  in_=x_d[0, :, ib, :, 0:9, :])
            for p in (0, 3, 1, 2):
                nc.sync.dma_start(out=w_sb[:, 0, 0, p], in_=w_d[:, 0, 0, p])
            bias_sb = cpool.tile([128, QB], f32)
            nc.sync.dma_start(out=bias_sb[:], in_=b_d[:])
            for p in (0, 3, 1, 2):
                nc.sync.dma_start(out=w_sb[:, 0, 1, p], in_=w_d[:, 0, 1, p])
            r0, r1 = chunks0[1]
            nc.sync.dma_start(out=xeo0[:, :, :, r0:r1, :],
                              in_=x_d[0, :, :, :, r0:r1, :])
            for ib in range(CB):
                nc.sync.dma_start(out=w_sb[:, 1, ib], in_=w_d[:, 1, ib])
            r0, r1 = chunks0[2]
            nc.sync.dma_start(out=xeo0[:, :, :, r0:r1, :],
                              in_=x_d[0, :, :, :, r0:r1, :])
            for (r0, r1) in chunks0[:2]:
                transform(nc, v0, xeo0, r0, r1)

            vs = [v0]
            xeo = xeo0
            for b in range(BPC):
                v_sb = vs[b]
                # prefetch next image's planes; transform ops are metered
                # out between groups (~5/group) so DVE stays just under PE
                pend = transform_ops(nc, v_sb, xeo, [(18, 34), (34, 50),
                                                     (50, 66)]) \
                    if b == 0 else pend_next
                if b + 1 < BPC:
                    xeo_n = xpool.tile([128, CB, 4, HP, NT], bf16, tag="xeo")
                    v_n = vpool.tile([128, CB, 4, HP, NT], bf16, tag="vv")
                    vs.append(v_n)
                    pend += transform_ops(nc, v_n, xeo_n, [(0, 9), (9, 18)])
                    pend_next = transform_ops(nc, v_n, xeo_n,
                                              [(18, 34), (34, 50), (50, 66)])
                    xeo = xeo_n

                def emit_group(cb, y0, nrow):
                    # split the 4 m-slots over two PSUM tiles: A holds m0/m3
                    # (released early by the ACT seeds), B holds m1/m2
                    # (released by the DVE adds) -- the next-next group's
                    # first 12 matmuls only need an A tile, so PE keeps
                    # running while DVE drains B
                    psa = ppool.tile([128, 2, NROW * NT], f32, tag="psa")
                    psb = ppool.tile([128, 2, NROW * NT], f32, tag="psb")
                    slot = {0: psa[:, 0], 3: psa[:, 1],
                            1: psb[:, 0], 2: psb[:, 1]}
                    for p in (0, 3, 1, 2):
                        i = 0
                        for ib in range(CB):
                            for kh in range(KS):
                                nc.tensor.matmul(
                                    slot[p][:, :nrow * NT],
                                    lhsT=w_sb[:, cb, ib, p, kh, :],
                                    rhs=v_sb[:, ib, p, y0 + kh:y0 + kh + nrow, :],
                                    start=(i == 0),
                                    stop=(i == CB * KS - 1),
                                )
                                i += 1
                    ot = opool.tile([128, NROW, W], f32, tag="ot")
                    ot = ot[:, :nrow]
                    ev = ot[:, :, 0:W:2]
                    od = ot[:, :, 1:W:2]
                    m = [slot[p][:, :nrow * NT]
                         .rearrange("p (r t) -> p r t", t=NT) for p in range(4)]
                    # Y_even = m0+m1+m2+bias ; Y_odd = m1-m2-m3+bias
                    nc.scalar.activation(
                        ev, m[0], mybir.ActivationFunctionType.Identity,
                        bias=bias_sb[:, cb:cb + 1], scale=1.0)
                    nc.scalar.activation(
                        od, m[3], mybir.ActivationFunctionType.Identity,
                        bias=bias_sb[:, cb:cb + 1], scale=-1.0)
                    nc.vector.tensor_add(ev, ev, m[1])
                    nc.vector.tensor_add(ev, ev, m[2])
                    nc.vector.tensor_add(od, od, m[1])
                    nc.vector.tensor_sub(od, od, m[2])
                    nc.sync.dma_start(out=o_d[b, cb, :, y0:y0 + nrow, :],
                                      in_=ot[:])

                first = (b == 0)
                last = (b == BPC - 1)
                # (rb, cb) order: V row-chunks are consumed progressively,
                # so metered transforms always land a group ahead of use
                for rb in range(NRB):
                    for cb in range(QB):
                        y0 = rb * NROW
                        if first and rb == 0 and cb == 0:
                            emit_group(cb, 0, NROW // 2)
                            for op in pend[:3]:
                                op()
                            del pend[:3]
                            emit_group(cb, NROW // 2, NROW // 2)
                        elif last and rb == NRB - 1 and cb == QB - 1:
                            emit_group(cb, y0, NROW // 2)
                            emit_group(cb, y0 + NROW // 2, NROW // 2)
                        else:
                            emit_group(cb, y0, NROW)
                        if b == 0 and rb == 0:
                            r0, r1 = chunks0[3 + cb]
                            nc.sync.dma_start(out=xeo0[:, :, :, r0:r1, :],
                                              in_=x_d[0, :, :, :, r0:r1, :])
                        if b + 1 < BPC and rb in (1, 2):
                            q0, q1 = [(0, 18), (18, 34), (34, 50), (50, 66)][
                                (rb - 1) * 2 + cb]
                            nc.sync.dma_start(out=xeo_n[:, :, :, q0:q1, :],
                                              in_=x_d[b + 1, :, :, :, q0:q1, :])
                        n = min(5, len(pend))
                        for op in pend[:n]:
                            op()
                        del pend[:n]
                for op in pend:
                    op()
    nc.compile()
    return nc


def prep_direct(x, weight, bias):
    import ml_dtypes
    xs = x.reshape(NCORES, BPC, CB, 128, H, W).transpose(0, 1, 3, 2, 4, 5)
    xs = np.pad(xs, [(0, 0), (0, 0), (0, 0), (0, 0), (1, 1), (1, 1)])
    xs = np.ascontiguousarray(xs).astype(ml_dtypes.bfloat16)
    # w[co, ci, kh, kw] -> [ci, ci_blk, kh, kw, co_blk, co]
    wp = weight.reshape(QB, 128, CB, 128, KS, KS).transpose(3, 2, 4, 5, 0, 1)
    wp = np.ascontiguousarray(wp).astype(ml_dtypes.bfloat16)
    bp = np.ascontiguousarray(bias.reshape(QB, 128).T).astype(np.float32)
    return [{"x": xs[c], "w": wp, "bias": bp} for c in range(NCORES)]


def prep_wino(x, weight, bias):
    import ml_dtypes
    NT = W // 2
    xs = x.reshape(NCORES, BPC, CB, 128, H, W).transpose(0, 1, 3, 2, 4, 5)
    xs = np.pad(xs, [(0, 0)] * 4 + [(1, 1), (1, 1)])  # [NC,BPC,128,CB,66,66]
    xe = xs[..., 0::2]                                 # cols 0,2,..,64  (33)
    xo = xs[..., 1::2]                                 # cols 1,3,..,65  (33)
    planes = np.stack([xe[..., 0:NT], xo[..., 0:NT],
                       xe[..., 1:NT + 1], xo[..., 1:NT + 1]], axis=4)
    # [NC, BPC, 128, CB, 4, 66, NT]
    xp = np.ascontiguousarray(planes).astype(ml_dtypes.bfloat16)
    # U[p] from g=w[..,kh,:]: U0=g0, U1=(g0+g1+g2)/2, U2=(g0-g1+g2)/2, U3=g2
    g = weight.astype(np.float64)
    u = np.stack([g[..., 0],
                  (g[..., 0] + g[..., 1] + g[..., 2]) * 0.5,
                  (g[..., 0] - g[..., 1] + g[..., 2]) * 0.5,
                  g[..., 2]], axis=-1)                 # [co, ci, kh, 4]
    up = u.reshape(QB, 128, CB, 128, KS, 4).transpose(3, 0, 2, 5, 4, 1)
    up = np.ascontiguousarray(up).astype(ml_dtypes.bfloat16)
    bp = np.ascontiguousarray(bias.reshape(QB, 128).T).astype(np.float32)
    return [{"x": xp[c], "w": up, "bias": bp} for c in range(NCORES)]


def build_program():
    return build_wino() if ALGO == "wino" else build_direct()


def prep_inputs(x, weight, bias):
    return (prep_wino if ALGO == "wino" else prep_direct)(x, weight, bias)


def kernel(x, weight, bias):
    global _prog
    from concourse.bass_utils import run_bass_kernel_spmd

    if _prog is None:
        _prog = build_program()
    in_maps = prep_inputs(np.asarray(x, dtype=np.float32),
                          np.asarray(weight, dtype=np.float32),
                          np.asarray(bias, dtype=np.float32))
    res = run_bass_kernel_spmd(_prog, in_maps, list(range(NCORES)))
    outs = [r["out"].reshape(BPC, COUT, H, W) for r in res.results]
    return np.concatenate(outs, axis=0).astype(np.float32)


# revision 28
# speedup vs baseline: 1.1215x; 1.0047x over previous
"""Conv2d 3x3 (B=32, 256->256 ch, 64x64, pad 1) on 8 trn2 NeuronCores.

Data-parallel over batch: 4 images per core, weight/bias replicated.

Two algorithms:
- 'direct': implicit GEMM, 9 taps x 2 ci-blocks of shifted matmuls
  accumulating in PSUM (bf16 inputs, fp32 accumulation).
- 'wino': 1-D Winograd F(2,3) along W. PE work drops 1.5x (6 effective
  taps instead of 9). Host deinterleaves the padded image into 4
  aligned column planes (xe, xo, xe1, xo1) so the DVE input transform
  runs in 2x packed-bf16 mode; the 4 Winograd m-slots accumulate in 4
  PSUM banks per group; the inverse transform is 2 ACT ops (m0+bias /
  -m3+bias, reading PSUM) + 4 in-place DVE adds.
"""

import numpy as np

B, CIN, COUT, H, W, KS = 32, 256, 256, 64, 64, 3
NCORES = 8
BPC = B // NCORES            # images per core
CB = CIN // 128              # input-channel blocks
QB = COUT // 128             # output-channel blocks

ALGO = "wino"

_prog = None


def _make_nc():
    from concourse import bacc

    return bacc.Bacc("TRN2", target_bir_lowering=False, debug=False)


def build_direct():
    import concourse.mybir as mybir
    from concourse.tile import TileContext

    bf16 = mybir.dt.bfloat16
    f32 = mybir.dt.float32
    NROW = 8                     # output rows per matmul group (N = 512)
    NRB = H // NROW

    nc = _make_nc()
    x_d = nc.dram_tensor("x", [BPC, 128, CB, H + 2, W + 2], bf16,
                         kind="ExternalInput").ap()
    w_d = nc.dram_tensor("w", [128, CB, KS, KS, QB, 128], bf16,
                         kind="ExternalInput").ap()
    b_d = nc.dram_tensor("bias", [128, QB], f32, kind="ExternalInput").ap()
    o_d = nc.dram_tensor("out", [BPC, QB, 128, H, W], f32, kind="ExternalOutput").ap()

    with TileContext(nc) as tc:
        with tc.tile_pool(name="const", bufs=1) as cpool, \
             tc.tile_pool(name="xpad", bufs=3) as xpool, \
             tc.tile_pool(name="ot", bufs=4) as opool, \
             tc.tile_pool(name="ps", bufs=4, space="PSUM") as ppool:
            # image 0 load first (critical path), chunked over rows
            xps = []
            row_chunks = [(0, 18), (18, 34), (34, 50), (50, 66)]
            xp0 = xpool.tile([128, CB, H + 2, W + 2], bf16, tag="xp")
            for (r0, r1) in row_chunks:
                nc.sync.dma_start(out=xp0[:, :, r0:r1, :],
                                  in_=x_d[0, :, :, r0:r1, :])
            xps.append(xp0)

            w_sb = cpool.tile([128, CB, KS, KS, QB, 128], bf16)
            nc.sync.dma_start(out=w_sb[:], in_=w_d[:])
            bias_sb = cpool.tile([128, QB], f32)
            nc.sync.dma_start(out=bias_sb[:], in_=b_d[:])

            for b in range(BPC):
                if b < len(xps):
                    xp = xps[b]
                else:
                    xp = xpool.tile([128, CB, H + 2, W + 2], bf16, tag="xp")
                    nc.sync.dma_start(out=xp[:], in_=x_d[b])

                for cb in range(QB):
                    for rb in range(NRB):
                        y0 = rb * NROW
                        ps = ppool.tile([128, NROW * W], f32)
                        n_mm = CB * KS * KS
                        i = 0
                        for ib in range(CB):
                            for kh in range(KS):
                                for kw in range(KS):
                                    nc.tensor.matmul(
                                        ps[:],
                                        lhsT=w_sb[:, ib, kh, kw, cb, :],
                                        rhs=xp[:, ib, y0 + kh:y0 + kh + NROW,
                                               kw:kw + W],
                                        start=(i == 0),
                                        stop=(i == n_mm - 1),
                                    )
                                    i += 1
                        ot = opool.tile([128, NROW * W], f32)
                        nc.vector.tensor_scalar_add(ot[:], ps[:],
                                                    bias_sb[:, cb:cb + 1])
                        nc.sync.dma_start(out=o_d[b, cb, :, y0:y0 + NROW, :],
                                          in_=ot[:])
    nc.compile()
    return nc


def build_wino():
    import concourse.mybir as mybir
    from concourse.tile import TileContext

    bf16 = mybir.dt.bfloat16
    f32 = mybir.dt.float32
    NROW = 16                    # output rows per group (N = 16*32 = 512)
    NRB = H // NROW              # 4 groups per (img, co-blk)
    NT = W // 2                  # 32 Winograd tiles per row
    HP = H + 2                   # padded rows

    nc = _make_nc()
    # 4 column planes per (img, blk): xe, xo, xe1, xo1 -- each [66 rows, 32]
    x_d = nc.dram_tensor("x", [BPC, 128, CB, 4, HP, NT], bf16,
                         kind="ExternalInput").ap()
    # Winograd-transformed weights U[p], p=0..3
    w_d = nc.dram_tensor("w", [128, QB, CB, 4, KS, 128], bf16,
                         kind="ExternalInput").ap()
    b_d = nc.dram_tensor("bias", [128, QB], f32, kind="ExternalInput").ap()
    o_d = nc.dram_tensor("out", [BPC, QB, 128, H, W], f32, kind="ExternalOutput").ap()

    def transform_ops(nc, v_sb, xeo, ranges):
        # planes: 0=xe, 1=xo, 2=xe1, 3=xo1. One thunk per DVE op so the
        # caller can meter them out between matmul groups.
        ops = []
        for (r0, r1) in ranges:
            rs = slice(r0, r1)
            for ib in range(CB):
                xe = xeo[:, ib, 0, rs]
                xo = xeo[:, ib, 1, rs]
                xe1 = xeo[:, ib, 2, rs]
                xo1 = xeo[:, ib, 3, rs]
                ops += [
                    lambda o=v_sb[:, ib, 0, rs], a=xe, b=xe1:
                        nc.vector.tensor_sub(o, a, b),          # V0=d0-d2
                    lambda o=v_sb[:, ib, 1, rs], a=xo, b=xe1:
                        nc.vector.tensor_add(o, a, b),          # V1=d1+d2
                    lambda o=v_sb[:, ib, 2, rs], a=xe1, b=xo:
                        nc.vector.tensor_sub(o, a, b),          # V2=d2-d1
                    lambda o=v_sb[:, ib, 3, rs], a=xo, b=xo1:
                        nc.vector.tensor_sub(o, a, b),          # V3=d1-d3
                ]
        return ops

    def transform(nc, v_sb, xeo, r0, r1):
        # emit in MM pos order (V0, V3, V1, V2 across both blocks) so the
        # first matmuls unblock after 2 ops instead of 8
        ops = transform_ops(nc, v_sb, xeo, [(r0, r1)])
        for i in (0, 4, 3, 7, 1, 5, 2, 6):
            ops[i]()

    with TileContext(nc) as tc:
        with tc.tile_pool(name="const", bufs=1) as cpool, \
             tc.tile_pool(name="xeo", bufs=2) as xpool, \
             tc.tile_pool(name="vv", bufs=2) as vpool, \
             tc.tile_pool(name="ot", bufs=6) as opool, \
             tc.tile_pool(name="ps", bufs=2, space="PSUM") as ppool:
            # image 0: DMA + transform in row chunks to shorten the
            # critical path to the first matmul group
            chunks = [(0, 18), (18, 34), (34, 50), (50, 66)]
            # weights first: cb=0 half gates the very first matmul group
            w_sb = cpool.tile([128, QB, CB, 4, KS, 128], bf16)
            chunks0 = [(0, 9), (9, 18), (18, 34), (34, 50), (50, 66)]
            xeo0 = xpool.tile([128, CB, 4, HP, NT], bf16, tag="xeo")
            v0 = vpool.tile([128, CB, 4, HP, NT], bf16, tag="vv")
            # critical-path order: x rows 0-9 per block, then w(cb0) pos
            # slices in MM order -- each lands on its own DMA queue so the
            # first matmuls start ~10us in instead of ~19us
            for ib in range(CB):
                nc.sync.dma_start(out=xeo0[:, ib, :, 0:9, :],
                                  in_=x_d[0, :, ib, :, 0:9, :])
            # w slices in exact first-group consumption order: each pos
            # chain contracts over BOTH ci blocks
            nc.sync.dma_start(out=w_sb[:, 0, 0, 0], in_=w_d[:, 0, 0, 0])
            nc.sync.dma_start(out=w_sb[:, 0, 1, 0], in_=w_d[:, 0, 1, 0])
            nc.sync.dma_start(out=w_sb[:, 0, 0, 3], in_=w_d[:, 0, 0, 3])
            nc.sync.dma_start(out=w_sb[:, 0, 1, 3], in_=w_d[:, 0, 1, 3])
            r0, r1 = chunks0[1]
            nc.sync.dma_start(out=xeo0[:, :, :, r0:r1, :],
                              in_=x_d[0, :, :, :, r0:r1, :])
            nc.sync.dma_start(out=w_sb[:, 0, 0, 1], in_=w_d[:, 0, 0, 1])
            nc.sync.dma_start(out=w_sb[:, 0, 1, 1], in_=w_d[:, 0, 1, 1])
            nc.sync.dma_start(out=w_sb[:, 0, 0, 2], in_=w_d[:, 0, 0, 2])
            nc.sync.dma_start(out=w_sb[:, 0, 1, 2], in_=w_d[:, 0, 1, 2])
            bias_sb = cpool.tile([128, QB], f32)
            nc.sync.dma_start(out=bias_sb[:], in_=b_d[:])
            r0, r1 = chunks0[2]
            nc.sync.dma_start(out=xeo0[:, :, :, r0:r1, :],
                              in_=x_d[0, :, :, :, r0:r1, :])
            for ib in range(CB):
                nc.sync.dma_start(out=w_sb[:, 1, ib], in_=w_d[:, 1, ib])
            for (r0, r1) in chunks0[:2]:
                transform(nc, v0, xeo0, r0, r1)

            vs = [v0]
            xeo = xeo0
            for b in range(BPC):
                v_sb = vs[b]
                # prefetch next image's planes; transform ops are metered
                # out between groups (~5/group) so DVE stays just under PE
                pend = transform_ops(nc, v_sb, xeo, [(18, 34), (34, 50),
                                                     (50, 66)]) \
                    if b == 0 else pend_next
                if b + 1 < BPC:
                    xeo_n = xpool.tile([128, CB, 4, HP, NT], bf16, tag="xeo")
                    v_n = vpool.tile([128, CB, 4, HP, NT], bf16, tag="vv")
                    vs.append(v_n)
                    pend += transform_ops(nc, v_n, xeo_n, [(0, 9), (9, 18)])
                    pend_next = transform_ops(nc, v_n, xeo_n,
                                              [(18, 34), (34, 50), (50, 66)])
                    xeo = xeo_n

                def emit_group(cb, y0, nrow):
                    # split the 4 m-slots over two PSUM tiles: A holds m0/m3
                    # (released early by the ACT seeds), B holds m1/m2
                    # (released by the DVE adds) -- the next-next group's
                    # first 12 matmuls only need an A tile, so PE keeps
                    # running while DVE drains B
                    psa = ppool.tile([128, 2, NROW * NT], f32, tag="psa")
                    psb = ppool.tile([128, 2, NROW * NT], f32, tag="psb")
                    slot = {0: psa[:, 0], 3: psa[:, 1],
                            1: psb[:, 0], 2: psb[:, 1]}
                    for p in (0, 3, 1, 2):
                        i = 0
                        for ib in range(CB):
                            for kh in range(KS):
                                nc.tensor.matmul(
                                    slot[p][:, :nrow * NT],
                                    lhsT=w_sb[:, cb, ib, p, kh, :],
                                    rhs=v_sb[:, ib, p, y0 + kh:y0 + kh + nrow, :],
                                    start=(i == 0),
                                    stop=(i == CB * KS - 1),
                                )
                                i += 1
                    ot = opool.tile([128, NROW, W], f32, tag="ot")
                    ot = ot[:, :nrow]
                    ev = ot[:, :, 0:W:2]
                    od = ot[:, :, 1:W:2]
                    m = [slot[p][:, :nrow * NT]
                         .rearrange("p (r t) -> p r t", t=NT) for p in range(4)]
                    # Y_even = m0+m1+m2+bias ; Y_odd = m1-m2-m3+bias
                    nc.scalar.activation(
                        ev, m[0], mybir.ActivationFunctionType.Identity,
                        bias=bias_sb[:, cb:cb + 1], scale=1.0)
                    nc.scalar.activation(
                        od, m[3], mybir.ActivationFunctionType.Identity,
                        bias=bias_sb[:, cb:cb + 1], scale=-1.0)
                    nc.vector.tensor_add(ev, ev, m[1])
                    nc.vector.tensor_add(ev, ev, m[2])
                    nc.vector.tensor_add(od, od, m[1])
                    nc.vector.tensor_sub(od, od, m[2])
                    nc.sync.dma_start(out=o_d[b, cb, :, y0:y0 + nrow, :],
                                      in_=ot[:])

                first = (b == 0)
                last = (b == BPC - 1)
                # (rb, cb) order: V row-chunks are consumed progressively,
                # so metered transforms always land a group ahead of use
                for rb in range(NRB):
                    for cb in range(QB):
                        y0 = rb * NROW
                        if first and rb == 0 and cb == 0:
                            emit_group(cb, 0, NROW // 2)
                            for op in pend[:3]:
                                op()
                            del pend[:3]
                            emit_group(cb, NROW // 2, NROW // 2)
                        elif last and rb == NRB - 1 and cb == QB - 1:
                            emit_group(cb, y0, NROW // 2)
                            emit_group(cb, y0 + NROW // 2, NROW // 2)
                        else:
                            emit_group(cb, y0, NROW)
                        if b == 0 and rb == 0:
                            r0, r1 = chunks0[3 + cb]
                            nc.sync.dma_start(out=xeo0[:, :, :, r0:r1, :],
                                              in_=x_d[0, :, :, :, r0:r1, :])
                        if b + 1 < BPC and rb in (1, 2):
                            q0, q1 = [(0, 18), (18, 34), (34, 50), (50, 66)][
                                (rb - 1) * 2 + cb]
                            nc.sync.dma_start(out=xeo_n[:, :, :, q0:q1, :],
                                              in_=x_d[b + 1, :, :, :, q0:q1, :])
                        n = min(5, len(pend))
                        for op in pend[:n]:
                            op()
                        del pend[:n]
                for op in pend:
                    op()
    nc.compile()
    return nc


def prep_direct(x, weight, bias):
    import ml_dtypes
    xs = x.reshape(NCORES, BPC, CB, 128, H, W).transpose(0, 1, 3, 2, 4, 5)
    xs = np.pad(xs, [(0, 0), (0, 0), (0, 0), (0, 0), (1, 1), (1, 1)])
    xs = np.ascontiguousarray(xs).astype(ml_dtypes.bfloat16)
    # w[co, ci, kh, kw] -> [ci, ci_blk, kh, kw, co_blk, co]
    wp = weight.reshape(QB, 128, CB, 128, KS, KS).transpose(3, 2, 4, 5, 0, 1)
    wp = np.ascontiguousarray(wp).astype(ml_dtypes.bfloat16)
    bp = np.ascontiguousarray(bias.reshape(QB, 128).T).astype(np.float32)
    return [{"x": xs[c], "w": wp, "bias": bp} for c in range(NCORES)]


def prep_wino(x, weight, bias):
    import ml_dtypes
    NT = W // 2
    xs = x.reshape(NCORES, BPC, CB, 128, H, W).transpose(0, 1, 3, 2, 4, 5)
    xs = np.pad(xs, [(0, 0)] * 4 + [(1, 1), (1, 1)])  # [NC,BPC,128,CB,66,66]
    xe = xs[..., 0::2]                                 # cols 0,2,..,64  (33)
    xo = xs[..., 1::2]                                 # cols 1,3,..,65  (33)
    planes = np.stack([xe[..., 0:NT], xo[..., 0:NT],
                       xe[..., 1:NT + 1], xo[..., 1:NT + 1]], axis=4)
    # [NC, BPC, 128, CB, 4, 66, NT]
    xp = np.ascontiguousarray(planes).astype(ml_dtypes.bfloat16)
    # U[p] from g=w[..,kh,:]: U0=g0, U1=(g0+g1+g2)/2, U2=(g0-g1+g2)/2, U3=g2
    g = weight.astype(np.float64)
    u = np.stack([g[..., 0],
                  (g[..., 0] + g[..., 1] + g[..., 2]) * 0.5,
                  (g[..., 0] - g[..., 1] + g[..., 2]) * 0.5,
                  g[..., 2]], axis=-1)                 # [co, ci, kh, 4]
    up = u.reshape(QB, 128, CB, 128, KS, 4).transpose(3, 0, 2, 5, 4, 1)
    up = np.ascontiguousarray(up).astype(ml_dtypes.bfloat16)
    bp = np.ascontiguousarray(bias.reshape(QB, 128).T).astype(np.float32)
    return [{"x": xp[c], "w": up, "bias": bp} for c in range(NCORES)]


def build_program():
    return build_wino() if ALGO == "wino" else build_direct()


def prep_inputs(x, weight, bias):
    return (prep_wino if ALGO == "wino" else prep_direct)(x, weight, bias)


def kernel(x, weight, bias):
    global _prog
    from concourse.bass_utils import run_bass_kernel_spmd

    if _prog is None:
        _prog = build_program()
    in_maps = prep_inputs(np.asarray(x, dtype=np.float32),
                          np.asarray(weight, dtype=np.float32),
                          np.asarray(bias, dtype=np.float32))
    res = run_bass_kernel_spmd(_prog, in_maps, list(range(NCORES)))
    outs = [r["out"].reshape(BPC, COUT, H, W) for r in res.results]
    return np.concatenate(outs, axis=0).astype(np.float32)
